# revision 7
# baseline (speedup 1.0000x reference)
"""ALaCarteClassifier Trainium2 kernel.

Model: embedding gather -> UNK substitution -> GRU(S=512,H=512) -> maxpool -> linear.
Sharding: data-parallel over batch (B=32) across 8 NeuronCores (4 rows/core).
Embedding table + weights replicated per core. No collectives.

Device pipeline per core (B_loc=4, T=2048 tokens, s-major token order t=s*4+b):
  1. indirect-DMA gather of fp16 table rows -> e [tok(part), 256]   (memory-bound part)
  2. PE-transpose e -> eT [e-dim(part), 2, T]; UNK fix as rank-1 update
     eT += induced (x) unkf  (one scalar_tensor_tensor per e-chunk)
  3. xiT[g, tok] = W_ih @ eT + (b_ih + b_hh baked for r,z; b_ih for n)  via PE
  4. GRU recurrence, 512 fully-unrolled steps; stationary fp16 W_hh tiles (FWL),
     moving hT [128,4]; gates in PSUM; running max-pool on DVE
  5. pooled @ W_proj.T + b_proj via PSUM accumulation (ones (x) b trick not needed:
     b added via DVE broadcast); DMA out [4, 2] f32
"""

import numpy as np

import concourse.bass as bass
import concourse.mybir as mybir
import concourse.tile as tile
from concourse import bacc
from concourse.bass_utils import run_bass_kernel_spmd
from concourse.masks import make_identity

# problem dims (hardcoded per harness rules)
VOCAB = 200000
E = 256
H = 512
B = 32
S = 512
C = 2
NCORES = 8
BL = B // NCORES          # 4 batch rows per core
T = BL * S                # 2048 tokens per core
TCH = T // 128            # 16 token chunks
ECH = E // 128            # 2 embedding-dim chunks
KCH = H // 128            # 4 hidden-dim chunks (GRU contraction)
MCH = 3 * H // 128        # 12 gate-row chunks (r:0-3, z:4-7, n:8-11)

F16 = mybir.dt.float16
F32 = mybir.dt.float32
I32 = mybir.dt.int32
AF = mybir.ActivationFunctionType
OP = mybir.AluOpType

# exposed for test.py
LAST_RESULT = None


def build_nc():
    nc = bacc.Bacc("TRN2", target_bir_lowering=False, debug=False, num_devices=NCORES)

    # ---- DRAM parameters (per-core shards / replicated weights) ----
    tab = nc.declare_dram_parameter("tab", [VOCAB + 1, E], F16, isOutput=False)
    tokp = nc.declare_dram_parameter("tokp", [128, TCH], I32, isOutput=False)
    unkf = nc.declare_dram_parameter("unkf", [128, T], F16, isOutput=False)
    wih = nc.declare_dram_parameter("wih", [E, 3 * H], F16, isOutput=False)
    whh = nc.declare_dram_parameter("whh", [H, 3 * H], F16, isOutput=False)
    bsum = nc.declare_dram_parameter("bsum", [128, MCH], F32, isOutput=False)
    bnrep = nc.declare_dram_parameter("bnrep", [128, 4 * BL], F32, isOutput=False)
    indt = nc.declare_dram_parameter("indt", [E, E], F16, isOutput=False)
    uvec = nc.declare_dram_parameter("uvec", [128, ECH], F16, isOutput=False)
    wproj = nc.declare_dram_parameter("wproj", [H, C], F16, isOutput=False)
    bproj = nc.declare_dram_parameter("bproj", [BL, C], F32, isOutput=False)
    out = nc.declare_dram_parameter("out", [BL, C], F32, isOutput=True)

    with tile.TileContext(nc) as tc:
        with (
            tc.tile_pool(name="persist", bufs=1) as pp,
            tc.tile_pool(name="gather", bufs=TCH) as gp,
            tc.tile_pool(name="tpsum", bufs=2, space="PSUM") as tps,
            tc.tile_pool(name="ipsum", bufs=1, space="PSUM") as ips,
            tc.tile_pool(name="xpsum", bufs=4, space="PSUM") as xps,
            tc.tile_pool(name="work", bufs=3) as wp,
        ):
            # ---------- load weights / metadata ----------
            wih_sb = pp.tile([128, ECH, 3 * H], F16, tag="wih")
            nc.sync.dma_start(out=wih_sb[:], in_=wih.rearrange("(c p) g -> p c g", p=128))
            whh_sb = pp.tile([128, KCH, 3 * H], F16, tag="whh")
            nc.sync.dma_start(out=whh_sb[:], in_=whh.rearrange("(c p) g -> p c g", p=128))
            tok_sb = pp.tile([128, TCH], I32, tag="tok")
            nc.sync.dma_start(out=tok_sb[:], in_=tokp[:])
            unkf_sb = pp.tile([128, T], F16, tag="unkf")
            nc.sync.dma_start(out=unkf_sb[:], in_=unkf[:])
            bsum_sb = pp.tile([128, MCH], F32, tag="bsum")
            nc.sync.dma_start(out=bsum_sb[:], in_=bsum[:])
            bnrep_sb = pp.tile([128, 4, BL], F32, tag="bnrep")
            nc.sync.dma_start(out=bnrep_sb[:], in_=bnrep.rearrange("p (c b) -> p c b", b=BL))
            indt_sb = pp.tile([128, ECH, E], F16, tag="indt")
            nc.sync.dma_start(out=indt_sb[:], in_=indt.rearrange("(c p) g -> p c g", p=128))
            uvec_sb = pp.tile([128, ECH], F16, tag="uvec")
            nc.sync.dma_start(out=uvec_sb[:], in_=uvec[:])
            wproj_sb = pp.tile([128, KCH, C], F16, tag="wproj")
            nc.sync.dma_start(out=wproj_sb[:], in_=wproj.rearrange("(c p) n -> p c n", p=128))
            bproj_sb = pp.tile([BL, C], F32, tag="bproj")
            nc.sync.dma_start(out=bproj_sb[:], in_=bproj[:])

            ident = pp.tile([128, 128], F16, tag="ident")
            make_identity(nc, ident[:])

            # ---------- induced = induction @ unk_vec  -> [128, ECH] fp16 cols ----------
            ind_ps = ips.tile([128, ECH], F32, tag="indps")
            for mc in range(ECH):
                for kc in range(ECH):
                    nc.tensor.matmul(
                        ind_ps[:, mc : mc + 1],
                        lhsT=indt_sb[:, kc, mc * 128 : (mc + 1) * 128],
                        rhs=uvec_sb[:, kc : kc + 1],
                        start=(kc == 0),
                        stop=(kc == ECH - 1),
                    )
            induced_sb = pp.tile([128, ECH], F16, tag="induced")
            nc.vector.tensor_copy(induced_sb[:], ind_ps[:])

            # ---------- gather + transpose -> eT [128, ECH, T] fp16 ----------
            eT = pp.tile([128, ECH, T], F16, tag="eT")
            for c in range(TCH):
                e_c = gp.tile([128, E], F16, tag="echunk")
                nc.gpsimd.indirect_dma_start(
                    out=e_c[:],
                    out_offset=None,
                    in_=tab[:],
                    in_offset=bass.IndirectOffsetOnAxis(ap=tok_sb[:, c : c + 1], axis=0),
                )
                for ec in range(ECH):
                    tp = tps.tile([128, 128], F16, tag="tp")
                    nc.tensor.transpose(
                        out=tp[:], in_=e_c[:, ec * 128 : (ec + 1) * 128], identity=ident[:]
                    )
                    nc.vector.tensor_copy(eT[:, ec, c * 128 : (c + 1) * 128], tp[:])

            # ---------- UNK rank-1 fix: eT += induced (x) unkf ----------
            for ec in range(ECH):
                nc.vector.scalar_tensor_tensor(
                    out=eT[:, ec, :],
                    in0=unkf_sb[:],
                    scalar=induced_sb[:, ec : ec + 1],
                    in1=eT[:, ec, :],
                    op0=OP.mult,
                    op1=OP.add,
                )

            # ---------- xiT = W_ih @ eT (+ biases) -> [128, MCH, T] fp16 ----------
            xiT = pp.tile([128, MCH, T], F16, tag="xiT")
            NT = 512
            for m in range(MCH):
                for q in range(T // NT):
                    xp = xps.tile([128, NT], F32, tag="xp")
                    for ec in range(ECH):
                        nc.tensor.matmul(
                            xp[:],
                            lhsT=wih_sb[:, ec, m * 128 : (m + 1) * 128],
                            rhs=eT[:, ec, q * NT : (q + 1) * NT],
                            start=(ec == 0),
                            stop=(ec == ECH - 1),
                        )
                    nc.vector.tensor_scalar_add(
                        xiT[:, m, q * NT : (q + 1) * NT], xp[:], bsum_sb[:, m : m + 1]
                    )

        # ---------- GRU recurrence (fully unrolled) ----------
        with (
            tc.tile_pool(name="gru_sb", bufs=3) as gsb,
            tc.tile_pool(name="h_pool", bufs=2) as hp,
            tc.tile_pool(name="rz_ps", bufs=2, space="PSUM") as rzp,
            tc.tile_pool(name="n_ps", bufs=2, space="PSUM") as nnp,
            tc.tile_pool(name="fin", bufs=1) as fin,
            tc.tile_pool(name="fin_ps", bufs=1, space="PSUM") as fps,
        ):
            hT = hp.tile([128, KCH, BL], F16, tag="hT")
            nc.gpsimd.memset(hT[:], 0.0)
            maxT = fin.tile([128, KCH, BL], F16, tag="maxT")
            nc.gpsimd.memset(maxT[:], -1.0e4)

            for t in range(S):
                rz_ps = rzp.tile([128, 8, BL], F32, tag="rz")
                n_ps = nnp.tile([128, 4, BL], F32, tag="n")
                for m in range(8):
                    for k in range(KCH):
                        nc.tensor.matmul(
                            rz_ps[:, m, :],
                            lhsT=whh_sb[:, k, m * 128 : (m + 1) * 128],
                            rhs=hT[:, k, :],
                            start=(k == 0),
                            stop=(k == KCH - 1),
                        )
                for m in range(8, 12):
                    for k in range(KCH):
                        nc.tensor.matmul(
                            n_ps[:, m - 8, :],
                            lhsT=whh_sb[:, k, m * 128 : (m + 1) * 128],
                            rhs=hT[:, k, :],
                            start=(k == 0),
                            stop=(k == KCH - 1),
                        )
                # gates
                rz_sb = gsb.tile([128, 8, BL], F16, tag="rz_sb")
                nc.vector.tensor_add(rz_sb[:], rz_ps[:], xiT[:, 0:8, t * BL : (t + 1) * BL])
                nc.scalar.activation(rz_sb[:], rz_sb[:], AF.Sigmoid)
                nb = gsb.tile([128, 4, BL], F32, tag="nb")
                nc.vector.tensor_add(nb[:], n_ps[:], bnrep_sb[:])
                nc.vector.tensor_mul(nb[:], nb[:], rz_sb[:, 0:4, :])
                nc.vector.tensor_add(nb[:], nb[:], xiT[:, 8:12, t * BL : (t + 1) * BL])
                n_sb = gsb.tile([128, 4, BL], F16, tag="n_sb")
                nc.scalar.activation(n_sb[:], nb[:], AF.Tanh)
                # h' = n + z * (h - n)
                d_sb = gsb.tile([128, 4, BL], F16, tag="d_sb")
                nc.vector.tensor_sub(d_sb[:], hT[:], n_sb[:])
                nc.vector.tensor_mul(d_sb[:], d_sb[:], rz_sb[:, 4:8, :])
                hT2 = hp.tile([128, KCH, BL], F16, tag="hT")
                nc.vector.tensor_add(hT2[:], n_sb[:], d_sb[:])
                nc.vector.tensor_max(maxT[:], maxT[:], hT2[:])
                hT = hT2

            # ---------- projection: out = pooled @ W_proj.T + b_proj ----------
            o_ps = fps.tile([BL, C], F32, tag="ops")
            for k in range(KCH):
                nc.tensor.matmul(
                    o_ps[:],
                    lhsT=maxT[:, k, :],
                    rhs=wproj_sb[:, k, :],
                    start=(k == 0),
                    stop=(k == KCH - 1),
                )
            o_sb = fin.tile([BL, C], F32, tag="osb")
            nc.vector.tensor_add(o_sb[:], o_ps[:], bproj_sb[:])
            nc.sync.dma_start(out=out[:], in_=o_sb[:])

    nc.compile()
    return nc


def _prep_inputs(x, emb_table, unk_vec, induction, W_ih, W_hh, b_ih, b_hh, W_proj, b_proj):
    """Host-side marshalling: shard over batch, pack layouts, cast to fp16."""
    x = np.asarray(x)
    tok = np.where(x == -1, VOCAB, x).astype(np.int32)       # [B, S]
    unk = (x == -1).astype(np.float16)                        # [B, S]

    tab16 = np.asarray(emb_table).astype(np.float16)          # [V+1, E]
    wih16 = np.asarray(W_ih).T.astype(np.float16).copy()      # [E, 3H]
    whh16 = np.asarray(W_hh).T.astype(np.float16).copy()      # [H, 3H]
    indt16 = np.asarray(induction).T.astype(np.float16).copy()  # [E, E] (k=j major)
    uv16 = np.asarray(unk_vec).astype(np.float16).reshape(ECH, 128).T.copy()  # [128, ECH]
    b_ih = np.asarray(b_ih).astype(np.float32)
    b_hh = np.asarray(b_hh).astype(np.float32)
    bihT = b_ih.reshape(MCH, 128).T                           # [128, 12]
    bhhT = b_hh.reshape(MCH, 128).T
    bsum = bihT.copy()
    bsum[:, 0:8] += bhhT[:, 0:8]                              # r,z: bake both biases
    bsum = np.ascontiguousarray(bsum, dtype=np.float32)
    bnrep = np.repeat(bhhT[:, 8:12, None], BL, axis=2).reshape(128, 4 * BL)
    bnrep = np.ascontiguousarray(bnrep, dtype=np.float32)
    wproj16 = np.asarray(W_proj).T.astype(np.float16).copy()  # [H, C]
    bproj32 = np.repeat(np.asarray(b_proj).astype(np.float32).reshape(1, C), BL, axis=0)

    shared = dict(
        tab=tab16, wih=wih16, whh=whh16, bsum=bsum, bnrep=bnrep,
        indt=indt16, uvec=uv16, wproj=wproj16, bproj=bproj32,
    )
    in_maps = []
    for i in range(NCORES):
        tok_i = tok[i * BL : (i + 1) * BL]                    # [BL, S]
        unk_i = unk[i * BL : (i + 1) * BL]
        tflat = tok_i.T.reshape(-1)                           # s-major, t = s*BL + b
        uflat = unk_i.T.reshape(-1)
        tokp = np.ascontiguousarray(tflat.reshape(TCH, 128).T, dtype=np.int32)
        unkf = np.ascontiguousarray(
            np.repeat(uflat[None, :], 128, axis=0), dtype=np.float16
        )
        in_maps.append(dict(shared, tokp=tokp, unkf=unkf))
    return in_maps


def _ensure_trace_hook():
    """Best-effort: make trace=True usable under axon.

    bass_utils fetches the NTFF hook from ``antenv.axon_hooks``; some agent
    images lack that module (boot degrades silently). Shim the registry and
    register the ctypes hook on libaxon_pjrt.so ourselves when possible.
    """
    import contextlib
    import ctypes
    import sys
    import types

    try:
        try:
            from antenv import axon_hooks  # noqa: PLC0415
        except ImportError:
            import antenv  # noqa: PLC0415

            axon_hooks = types.ModuleType("antenv.axon_hooks")
            _hook_box = [None]
            axon_hooks.set_axon_ntff_profile_hook = lambda h: _hook_box.__setitem__(0, h)
            axon_hooks.get_axon_ntff_profile_hook = lambda: _hook_box[0]
            sys.modules["antenv.axon_hooks"] = axon_hooks
            antenv.axon_hooks = axon_hooks
        if axon_hooks.get_axon_ntff_profile_hook() is not None:
            return True
        so_path = "/opt/axon/libaxon_pjrt.so"
        lib = ctypes.CDLL(so_path)
        if not hasattr(lib, "axon_start_nrt_profile"):
            return False
        lib.axon_start_nrt_profile.argtypes = [
            ctypes.POINTER(ctypes.c_int64),
            ctypes.c_size_t,
        ]
        lib.axon_start_nrt_profile.restype = ctypes.c_int64
        lib.axon_stop_nrt_profile.argtypes = [ctypes.c_char_p]
        lib.axon_stop_nrt_profile.restype = ctypes.c_int64

        @contextlib.contextmanager
        def _hook(output_dir, device_ids):
            import jax  # noqa: PLC0415

            jax.devices()
            if device_ids:
                ids = (ctypes.c_int64 * len(device_ids))(*device_ids)
                rc = lib.axon_start_nrt_profile(ids, len(device_ids))
            else:
                rc = lib.axon_start_nrt_profile(None, 0)
            if rc != 0:
                raise RuntimeError(f"axon_start_nrt_profile rc={rc}")
            try:
                yield
            finally:
                n = lib.axon_stop_nrt_profile(str(output_dir).encode())
                if n < 0:
                    raise RuntimeError(f"axon_stop_nrt_profile rc={n}")

        axon_hooks.set_axon_ntff_profile_hook(_hook)
        return True
    except Exception:
        return False


def kernel(**inputs):
    global LAST_RESULT
    import os

    nc = build_nc()
    in_maps = _prep_inputs(**inputs)
    trace = os.environ.get("KERNEL_TRACE", "1") == "1"
    if trace:
        trace = _ensure_trace_hook()
    core_ids = list(range(NCORES))
    try:
        res = run_bass_kernel_spmd(nc, in_maps, core_ids=core_ids, trace=trace)
    except Exception:
        if not trace:
            raise
        res = run_bass_kernel_spmd(nc, in_maps, core_ids=core_ids, trace=False)
    LAST_RESULT = res
    out = np.concatenate([r["out"] for r in res.results], axis=0)  # [B, C]
    return out.astype(np.float32)


# revision 9
# speedup vs baseline: 1.2564x; 1.2564x over previous
"""ALaCarteClassifier Trainium2 kernel.

Model: embedding gather -> UNK substitution -> GRU(S=512,H=512) -> maxpool -> linear.
Sharding: data-parallel over batch (B=32) across 8 NeuronCores (4 rows/core).
Embedding table + weights replicated per core. No collectives.

Device pipeline per core (B_loc=4, T=2048 tokens, s-major token order t=s*4+b):
  1. indirect-DMA gather of fp16 table rows -> e [tok(part), 256]   (memory-bound part)
  2. PE-transpose e -> eT [e-dim(part), 2, T]; UNK fix as rank-1 update
     eT += induced (x) unkf  (one scalar_tensor_tensor per e-chunk)
  3. xiT[g, tok] = W_ih @ eT + (b_ih + b_hh baked for r,z; b_ih for n)  via PE
  4. GRU recurrence, 512 fully-unrolled steps; stationary fp16 W_hh tiles (FWL),
     moving hT [128,4]; gates in PSUM; running max-pool on DVE
  5. pooled @ W_proj.T + b_proj via PSUM accumulation (ones (x) b trick not needed:
     b added via DVE broadcast); DMA out [4, 2] f32
"""

import numpy as np

import concourse.bass as bass
import concourse.mybir as mybir
import concourse.tile as tile
from concourse import bacc
from concourse.bass_utils import run_bass_kernel_spmd
from concourse.masks import make_identity

# problem dims (hardcoded per harness rules)
VOCAB = 200000
E = 256
H = 512
B = 32
S = 512
C = 2
NCORES = 8
BL = B // NCORES          # 4 batch rows per core
T = BL * S                # 2048 tokens per core
TCH = T // 128            # 16 token chunks
ECH = E // 128            # 2 embedding-dim chunks
KCH = H // 128            # 4 hidden-dim chunks (GRU contraction)
MCH = 3 * H // 128        # 12 gate-row chunks (r:0-3, z:4-7, n:8-11)

F16 = mybir.dt.float16
F32 = mybir.dt.float32
I32 = mybir.dt.int32
AF = mybir.ActivationFunctionType
OP = mybir.AluOpType

# exposed for test.py
LAST_RESULT = None


def build_nc():
    nc = bacc.Bacc("TRN2", target_bir_lowering=False, debug=False, num_devices=NCORES)

    # ---- DRAM parameters (per-core shards / replicated weights) ----
    tab = nc.declare_dram_parameter("tab", [VOCAB + 1, E], F16, isOutput=False)
    tokp = nc.declare_dram_parameter("tokp", [128, TCH], I32, isOutput=False)
    unkf = nc.declare_dram_parameter("unkf", [128, T], F16, isOutput=False)
    wih = nc.declare_dram_parameter("wih", [E, 3 * H], F16, isOutput=False)
    whh = nc.declare_dram_parameter("whh", [H, 3 * H], F16, isOutput=False)
    bsum = nc.declare_dram_parameter("bsum", [128, MCH], F32, isOutput=False)
    bnrep = nc.declare_dram_parameter("bnrep", [128, 4 * BL], F32, isOutput=False)
    indt = nc.declare_dram_parameter("indt", [E, E], F16, isOutput=False)
    uvec = nc.declare_dram_parameter("uvec", [128, ECH], F16, isOutput=False)
    wproj = nc.declare_dram_parameter("wproj", [H, C], F16, isOutput=False)
    bproj = nc.declare_dram_parameter("bproj", [BL, C], F32, isOutput=False)
    out = nc.declare_dram_parameter("out", [BL, C], F32, isOutput=True)

    with tile.TileContext(nc) as tc:
        with (
            tc.tile_pool(name="persist", bufs=1) as pp,
            tc.tile_pool(name="gather", bufs=TCH) as gp,
            tc.tile_pool(name="tpsum", bufs=2, space="PSUM") as tps,
            tc.tile_pool(name="ipsum", bufs=1, space="PSUM") as ips,
            tc.tile_pool(name="xpsum", bufs=4, space="PSUM") as xps,
            tc.tile_pool(name="work", bufs=3) as wp,
        ):
            # ---------- load weights / metadata ----------
            wih_sb = pp.tile([128, ECH, 3 * H], F16, tag="wih")
            nc.sync.dma_start(out=wih_sb[:], in_=wih.rearrange("(c p) g -> p c g", p=128))
            whh_sb = pp.tile([128, KCH, 3 * H], F16, tag="whh")
            nc.sync.dma_start(out=whh_sb[:], in_=whh.rearrange("(c p) g -> p c g", p=128))
            tok_sb = pp.tile([128, TCH], I32, tag="tok")
            nc.sync.dma_start(out=tok_sb[:], in_=tokp[:])
            unkf_sb = pp.tile([128, T], F16, tag="unkf")
            nc.sync.dma_start(out=unkf_sb[:], in_=unkf[:])
            bsum_sb = pp.tile([128, MCH], F32, tag="bsum")
            nc.sync.dma_start(out=bsum_sb[:], in_=bsum[:])
            bnrep_sb = pp.tile([128, 4, BL], F32, tag="bnrep")
            nc.sync.dma_start(out=bnrep_sb[:], in_=bnrep.rearrange("p (c b) -> p c b", b=BL))
            indt_sb = pp.tile([128, ECH, E], F16, tag="indt")
            nc.sync.dma_start(out=indt_sb[:], in_=indt.rearrange("(c p) g -> p c g", p=128))
            uvec_sb = pp.tile([128, ECH], F16, tag="uvec")
            nc.sync.dma_start(out=uvec_sb[:], in_=uvec[:])
            wproj_sb = pp.tile([128, KCH, C], F16, tag="wproj")
            nc.sync.dma_start(out=wproj_sb[:], in_=wproj.rearrange("(c p) n -> p c n", p=128))
            bproj_sb = pp.tile([BL, C], F32, tag="bproj")
            nc.sync.dma_start(out=bproj_sb[:], in_=bproj[:])

            ident = pp.tile([128, 128], F16, tag="ident")
            make_identity(nc, ident[:])

            # ---------- induced = induction @ unk_vec  -> [128, ECH] fp16 cols ----------
            ind_ps = ips.tile([128, ECH], F32, tag="indps")
            for mc in range(ECH):
                for kc in range(ECH):
                    nc.tensor.matmul(
                        ind_ps[:, mc : mc + 1],
                        lhsT=indt_sb[:, kc, mc * 128 : (mc + 1) * 128],
                        rhs=uvec_sb[:, kc : kc + 1],
                        start=(kc == 0),
                        stop=(kc == ECH - 1),
                    )
            induced_sb = pp.tile([128, ECH], F16, tag="induced")
            nc.vector.tensor_copy(induced_sb[:], ind_ps[:])

            # ---------- gather + transpose -> eT [128, ECH, T] fp16 ----------
            eT = pp.tile([128, ECH, T], F16, tag="eT")
            for c in range(TCH):
                e_c = gp.tile([128, E], F16, tag="echunk")
                nc.gpsimd.indirect_dma_start(
                    out=e_c[:],
                    out_offset=None,
                    in_=tab[:],
                    in_offset=bass.IndirectOffsetOnAxis(ap=tok_sb[:, c : c + 1], axis=0),
                )
                for ec in range(ECH):
                    tp = tps.tile([128, 128], F16, tag="tp")
                    nc.tensor.transpose(
                        out=tp[:], in_=e_c[:, ec * 128 : (ec + 1) * 128], identity=ident[:]
                    )
                    nc.vector.tensor_copy(eT[:, ec, c * 128 : (c + 1) * 128], tp[:])

            # ---------- UNK rank-1 fix: eT += induced (x) unkf ----------
            for ec in range(ECH):
                nc.vector.scalar_tensor_tensor(
                    out=eT[:, ec, :],
                    in0=unkf_sb[:],
                    scalar=induced_sb[:, ec : ec + 1],
                    in1=eT[:, ec, :],
                    op0=OP.mult,
                    op1=OP.add,
                )

            # ---------- xiT = W_ih @ eT (+ biases) -> [128, MCH, T] fp16 ----------
            xiT = pp.tile([128, MCH, T], F16, tag="xiT")
            NT = 512
            for m in range(MCH):
                for q in range(T // NT):
                    xp = xps.tile([128, NT], F32, tag="xp")
                    for ec in range(ECH):
                        nc.tensor.matmul(
                            xp[:],
                            lhsT=wih_sb[:, ec, m * 128 : (m + 1) * 128],
                            rhs=eT[:, ec, q * NT : (q + 1) * NT],
                            start=(ec == 0),
                            stop=(ec == ECH - 1),
                        )
                    nc.vector.tensor_scalar_add(
                        xiT[:, m, q * NT : (q + 1) * NT], xp[:], bsum_sb[:, m : m + 1]
                    )

        # ---------- GRU recurrence (fully unrolled) ----------
        with (
            tc.tile_pool(name="gru_sb", bufs=3) as gsb,
            tc.tile_pool(name="h_pool", bufs=2) as hp,
            tc.tile_pool(name="rz_ps", bufs=2, space="PSUM") as rzp,
            tc.tile_pool(name="n_ps", bufs=2, space="PSUM") as nnp,
            tc.tile_pool(name="z_ps", bufs=2, space="PSUM") as zzp,
            tc.tile_pool(name="fin", bufs=1) as fin,
            tc.tile_pool(name="fin_ps", bufs=1, space="PSUM") as fps,
        ):
            # State runs in g-space: g = h + 1 (corrections host-baked into
            # bsum/bnrep/bproj). n-gate weights are host-scaled by 2 so
            # tanh(x) = 2*sigmoid(2x) - 1 merges into the z sigmoid.
            hT = hp.tile([128, KCH, BL], F16, tag="hT")
            nc.gpsimd.memset(hT[:], 1.0)
            maxT = fin.tile([128, KCH, BL], F16, tag="maxT")
            nc.gpsimd.memset(maxT[:], -1.0e4)

            for t in range(S):
                r_ps = rzp.tile([128, 4, BL], F32, tag="r")
                n_ps = nnp.tile([128, 4, BL], F32, tag="n")
                z_ps = zzp.tile([128, 4, BL], F32, tag="z")
                # PE order r -> n -> z: the r and n gate chains hide under
                # the remaining matmuls; only z's chain trails the PE block.
                for m in range(4):
                    for k in range(KCH):
                        nc.tensor.matmul(
                            r_ps[:, m, :],
                            lhsT=whh_sb[:, k, m * 128 : (m + 1) * 128],
                            rhs=hT[:, k, :],
                            start=(k == 0),
                            stop=(k == KCH - 1),
                        )
                for m in range(8, 12):
                    for k in range(KCH):
                        nc.tensor.matmul(
                            n_ps[:, m - 8, :],
                            lhsT=whh_sb[:, k, m * 128 : (m + 1) * 128],
                            rhs=hT[:, k, :],
                            start=(k == 0),
                            stop=(k == KCH - 1),
                        )
                for m in range(4, 8):
                    for k in range(KCH):
                        nc.tensor.matmul(
                            z_ps[:, m - 4, :],
                            lhsT=whh_sb[:, k, m * 128 : (m + 1) * 128],
                            rhs=hT[:, k, :],
                            start=(k == 0),
                            stop=(k == KCH - 1),
                        )
                # r chain (hidden under n/z matmuls)
                rpre = gsb.tile([128, 4, BL], F32, tag="rpre")
                nc.vector.tensor_add(rpre[:], r_ps[:], xiT[:, 0:4, t * BL : (t + 1) * BL])
                r_s = gsb.tile([128, 4, BL], F16, tag="r_s")
                nc.scalar.activation(r_s[:], rpre[:], AF.Sigmoid)
                # n chain: nb = 2*(hn + bhn) * r  (hidden under z matmuls)
                nb = gsb.tile([128, 4, BL], F32, tag="nb")
                nc.vector.tensor_add(nb[:], n_ps[:], bnrep_sb[:])
                nc.vector.tensor_mul(nb[:], nb[:], r_s[:])
                # merged sigmoid input: [z_pre | 2*n_pre]
                zn = gsb.tile([128, 8, BL], F32, tag="zn")
                nc.vector.tensor_add(zn[:, 4:8, :], nb[:], xiT[:, 8:12, t * BL : (t + 1) * BL])
                nc.vector.tensor_add(zn[:, 0:4, :], z_ps[:], xiT[:, 4:8, t * BL : (t + 1) * BL])
                s_sb = gsb.tile([128, 8, BL], F16, tag="s_sb")
                nc.scalar.activation(s_sb[:], zn[:], AF.Sigmoid)
                # g' = 2*s_n + s_z*(g - 2*s_n)
                d_sb = gsb.tile([128, 4, BL], F16, tag="d_sb")
                nc.vector.scalar_tensor_tensor(
                    out=d_sb[:], in0=s_sb[:, 4:8, :], scalar=-2.0, in1=hT[:],
                    op0=OP.mult, op1=OP.add,
                )
                nc.vector.tensor_mul(d_sb[:], d_sb[:], s_sb[:, 0:4, :])
                hT2 = hp.tile([128, KCH, BL], F16, tag="hT")
                nc.vector.scalar_tensor_tensor(
                    out=hT2[:], in0=s_sb[:, 4:8, :], scalar=2.0, in1=d_sb[:],
                    op0=OP.mult, op1=OP.add,
                )
                nc.vector.tensor_max(maxT[:], maxT[:], hT2[:])
                hT = hT2

            # ---------- projection: out = pooled @ W_proj.T + b_proj ----------
            o_ps = fps.tile([BL, C], F32, tag="ops")
            for k in range(KCH):
                nc.tensor.matmul(
                    o_ps[:],
                    lhsT=maxT[:, k, :],
                    rhs=wproj_sb[:, k, :],
                    start=(k == 0),
                    stop=(k == KCH - 1),
                )
            o_sb = fin.tile([BL, C], F32, tag="osb")
            nc.vector.tensor_add(o_sb[:], o_ps[:], bproj_sb[:])
            nc.sync.dma_start(out=out[:], in_=o_sb[:])

    nc.compile()
    return nc


def _prep_inputs(x, emb_table, unk_vec, induction, W_ih, W_hh, b_ih, b_hh, W_proj, b_proj):
    """Host-side marshalling: shard over batch, pack layouts, cast to fp16."""
    x = np.asarray(x)
    tok = np.where(x == -1, VOCAB, x).astype(np.int32)       # [B, S]
    unk = (x == -1).astype(np.float16)                        # [B, S]

    tab16 = np.asarray(emb_table).astype(np.float16)          # [V+1, E]
    W_ih = np.asarray(W_ih).astype(np.float32)
    W_hh = np.asarray(W_hh).astype(np.float32)
    wih_s = W_ih.copy()
    wih_s[2 * H :, :] *= 2.0                                  # n gate x2 (tanh->sigmoid)
    whh_s = W_hh.copy()
    whh_s[2 * H :, :] *= 2.0
    wih16 = wih_s.T.astype(np.float16).copy()                 # [E, 3H]
    whh16 = whh_s.T.astype(np.float16).copy()                 # [H, 3H]
    whh_rowsum = W_hh.sum(axis=1).astype(np.float32)          # (W_hh @ 1)[g], unscaled
    indt16 = np.asarray(induction).T.astype(np.float16).copy()  # [E, E] (k=j major)
    uv16 = np.asarray(unk_vec).astype(np.float16).reshape(ECH, 128).T.copy()  # [128, ECH]
    b_ih = np.asarray(b_ih).astype(np.float32)
    b_hh = np.asarray(b_hh).astype(np.float32)
    bihT = b_ih.reshape(MCH, 128).T                           # [128, 12]
    bhhT = b_hh.reshape(MCH, 128).T
    csT = whh_rowsum.reshape(MCH, 128).T                      # g-space correction
    bsum = bihT.copy()
    bsum[:, 0:8] += bhhT[:, 0:8] - csT[:, 0:8]                # r,z: biases - W_hh@1
    bsum[:, 8:12] *= 2.0                                      # n: 2*b_ih (no b_hh here)
    bsum = np.ascontiguousarray(bsum, dtype=np.float32)
    bn = 2.0 * (bhhT[:, 8:12] - csT[:, 8:12])                 # n: 2*(b_hh - W_hh@1)
    bnrep = np.repeat(bn[:, :, None], BL, axis=2).reshape(128, 4 * BL)
    bnrep = np.ascontiguousarray(bnrep, dtype=np.float32)
    W_proj = np.asarray(W_proj).astype(np.float32)
    wproj16 = W_proj.T.astype(np.float16).copy()              # [H, C]
    bp = np.asarray(b_proj).astype(np.float32).reshape(1, C) - W_proj.sum(axis=1)[None, :]
    bproj32 = np.repeat(bp, BL, axis=0)                       # b - W_proj@1 (pooled in g-space)

    shared = dict(
        tab=tab16, wih=wih16, whh=whh16, bsum=bsum, bnrep=bnrep,
        indt=indt16, uvec=uv16, wproj=wproj16, bproj=bproj32,
    )
    in_maps = []
    for i in range(NCORES):
        tok_i = tok[i * BL : (i + 1) * BL]                    # [BL, S]
        unk_i = unk[i * BL : (i + 1) * BL]
        tflat = tok_i.T.reshape(-1)                           # s-major, t = s*BL + b
        uflat = unk_i.T.reshape(-1)
        tokp = np.ascontiguousarray(tflat.reshape(TCH, 128).T, dtype=np.int32)
        unkf = np.ascontiguousarray(
            np.repeat(uflat[None, :], 128, axis=0), dtype=np.float16
        )
        in_maps.append(dict(shared, tokp=tokp, unkf=unkf))
    return in_maps


def _ensure_trace_hook():
    """Best-effort: make trace=True usable under axon.

    bass_utils fetches the NTFF hook from ``antenv.axon_hooks``; some agent
    images lack that module (boot degrades silently). Shim the registry and
    register the ctypes hook on libaxon_pjrt.so ourselves when possible.
    """
    import contextlib
    import ctypes
    import sys
    import types

    try:
        try:
            from antenv import axon_hooks  # noqa: PLC0415
        except ImportError:
            import antenv  # noqa: PLC0415

            axon_hooks = types.ModuleType("antenv.axon_hooks")
            _hook_box = [None]
            axon_hooks.set_axon_ntff_profile_hook = lambda h: _hook_box.__setitem__(0, h)
            axon_hooks.get_axon_ntff_profile_hook = lambda: _hook_box[0]
            sys.modules["antenv.axon_hooks"] = axon_hooks
            antenv.axon_hooks = axon_hooks
        if axon_hooks.get_axon_ntff_profile_hook() is not None:
            return True
        so_path = "/opt/axon/libaxon_pjrt.so"
        lib = ctypes.CDLL(so_path)
        if not hasattr(lib, "axon_start_nrt_profile"):
            return False
        lib.axon_start_nrt_profile.argtypes = [
            ctypes.POINTER(ctypes.c_int64),
            ctypes.c_size_t,
        ]
        lib.axon_start_nrt_profile.restype = ctypes.c_int64
        lib.axon_stop_nrt_profile.argtypes = [ctypes.c_char_p]
        lib.axon_stop_nrt_profile.restype = ctypes.c_int64

        @contextlib.contextmanager
        def _hook(output_dir, device_ids):
            import jax  # noqa: PLC0415

            jax.devices()
            if device_ids:
                ids = (ctypes.c_int64 * len(device_ids))(*device_ids)
                rc = lib.axon_start_nrt_profile(ids, len(device_ids))
            else:
                rc = lib.axon_start_nrt_profile(None, 0)
            if rc != 0:
                raise RuntimeError(f"axon_start_nrt_profile rc={rc}")
            try:
                yield
            finally:
                n = lib.axon_stop_nrt_profile(str(output_dir).encode())
                if n < 0:
                    raise RuntimeError(f"axon_stop_nrt_profile rc={n}")

        axon_hooks.set_axon_ntff_profile_hook(_hook)
        return True
    except Exception:
        return False


def kernel(**inputs):
    global LAST_RESULT
    import os

    nc = build_nc()
    in_maps = _prep_inputs(**inputs)
    trace = os.environ.get("KERNEL_TRACE", "1") == "1"
    if trace:
        trace = _ensure_trace_hook()
    core_ids = list(range(NCORES))
    try:
        res = run_bass_kernel_spmd(nc, in_maps, core_ids=core_ids, trace=trace)
    except Exception:
        if not trace:
            raise
        res = run_bass_kernel_spmd(nc, in_maps, core_ids=core_ids, trace=False)
    LAST_RESULT = res
    out = np.concatenate([r["out"] for r in res.results], axis=0)  # [B, C]
    return out.astype(np.float32)


# revision 11
# speedup vs baseline: 1.2745x; 1.0144x over previous
"""ALaCarteClassifier Trainium2 kernel.

Model: embedding gather -> UNK substitution -> GRU(S=512,H=512) -> maxpool -> linear.
Sharding: data-parallel over batch (B=32) across 8 NeuronCores (4 rows/core).
Embedding table + weights replicated per core. No collectives.

Device pipeline per core (B_loc=4, T=2048 tokens, s-major token order t=s*4+b):
  1. indirect-DMA gather of fp16 table rows -> e [tok(part), 256]   (memory-bound part)
  2. PE-transpose e -> eT [e-dim(part), 2, T]; UNK fix as rank-1 update
     eT += induced (x) unkf  (one scalar_tensor_tensor per e-chunk)
  3. xiT[g, tok] = W_ih @ eT + (b_ih + b_hh baked for r,z; b_ih for n)  via PE
  4. GRU recurrence, 512 fully-unrolled steps; stationary fp16 W_hh tiles (FWL),
     moving hT [128,4]; gates in PSUM; running max-pool on DVE
  5. pooled @ W_proj.T + b_proj via PSUM accumulation (ones (x) b trick not needed:
     b added via DVE broadcast); DMA out [4, 2] f32
"""

import numpy as np

import concourse.bass as bass
import concourse.mybir as mybir
import concourse.tile as tile
from concourse import bacc
from concourse.bass_utils import run_bass_kernel_spmd
from concourse.masks import make_identity
from concourse.tile_rust import add_dep_helper

# problem dims (hardcoded per harness rules)
VOCAB = 200000
E = 256
H = 512
B = 32
S = 512
C = 2
NCORES = 8
BL = B // NCORES          # 4 batch rows per core
T = BL * S                # 2048 tokens per core
TCH = T // 128            # 16 token chunks
ECH = E // 128            # 2 embedding-dim chunks
KCH = H // 128            # 4 hidden-dim chunks (GRU contraction)
MCH = 3 * H // 128        # 12 gate-row chunks (r:0-3, z:4-7, n:8-11)

F16 = mybir.dt.float16
F32 = mybir.dt.float32
I32 = mybir.dt.int32
AF = mybir.ActivationFunctionType
OP = mybir.AluOpType

# exposed for test.py
LAST_RESULT = None


def build_nc():
    nc = bacc.Bacc("TRN2", target_bir_lowering=False, debug=False, num_devices=NCORES)

    # ---- DRAM parameters (per-core shards / replicated weights) ----
    tab = nc.declare_dram_parameter("tab", [VOCAB + 1, E], F16, isOutput=False)
    tokp = nc.declare_dram_parameter("tokp", [128, TCH], I32, isOutput=False)
    unkf = nc.declare_dram_parameter("unkf", [128, T], F16, isOutput=False)
    wih = nc.declare_dram_parameter("wih", [E, 3 * H], F16, isOutput=False)
    whh = nc.declare_dram_parameter("whh", [H, 3 * H], F16, isOutput=False)
    bsum = nc.declare_dram_parameter("bsum", [128, MCH], F32, isOutput=False)
    bnrep = nc.declare_dram_parameter("bnrep", [128, 4 * BL], F32, isOutput=False)
    indt = nc.declare_dram_parameter("indt", [E, E], F16, isOutput=False)
    uvec = nc.declare_dram_parameter("uvec", [128, ECH], F16, isOutput=False)
    wproj = nc.declare_dram_parameter("wproj", [H, C], F16, isOutput=False)
    bproj = nc.declare_dram_parameter("bproj", [BL, C], F32, isOutput=False)
    out = nc.declare_dram_parameter("out", [BL, C], F32, isOutput=True)

    with tile.TileContext(nc) as tc:
        with (
            tc.tile_pool(name="persist", bufs=1) as pp,
            tc.tile_pool(name="gather", bufs=TCH) as gp,
            tc.tile_pool(name="tpsum", bufs=2, space="PSUM") as tps,
            tc.tile_pool(name="ipsum", bufs=1, space="PSUM") as ips,
            tc.tile_pool(name="xpsum", bufs=4, space="PSUM") as xps,
            tc.tile_pool(name="work", bufs=3) as wp,
        ):
            # ---------- load weights / metadata ----------
            wih_sb = pp.tile([128, ECH, 3 * H], F16, tag="wih")
            nc.sync.dma_start(out=wih_sb[:], in_=wih.rearrange("(c p) g -> p c g", p=128))
            whh_sb = pp.tile([128, KCH, 3 * H], F16, tag="whh")
            nc.sync.dma_start(out=whh_sb[:], in_=whh.rearrange("(c p) g -> p c g", p=128))
            tok_sb = pp.tile([128, TCH], I32, tag="tok")
            nc.sync.dma_start(out=tok_sb[:], in_=tokp[:])
            unkf_sb = pp.tile([128, T], F16, tag="unkf")
            nc.sync.dma_start(out=unkf_sb[:], in_=unkf[:])
            bsum_sb = pp.tile([128, MCH], F32, tag="bsum")
            nc.sync.dma_start(out=bsum_sb[:], in_=bsum[:])
            bnrep_sb = pp.tile([128, 4, BL], F32, tag="bnrep")
            nc.sync.dma_start(out=bnrep_sb[:], in_=bnrep.rearrange("p (c b) -> p c b", b=BL))
            indt_sb = pp.tile([128, ECH, E], F16, tag="indt")
            nc.sync.dma_start(out=indt_sb[:], in_=indt.rearrange("(c p) g -> p c g", p=128))
            uvec_sb = pp.tile([128, ECH], F16, tag="uvec")
            nc.sync.dma_start(out=uvec_sb[:], in_=uvec[:])
            wproj_sb = pp.tile([128, KCH, C], F16, tag="wproj")
            nc.sync.dma_start(out=wproj_sb[:], in_=wproj.rearrange("(c p) n -> p c n", p=128))
            bproj_sb = pp.tile([BL, C], F32, tag="bproj")
            nc.sync.dma_start(out=bproj_sb[:], in_=bproj[:])

            ident = pp.tile([128, 128], F16, tag="ident")
            make_identity(nc, ident[:])

            # ---------- induced = induction @ unk_vec  -> [128, ECH] fp16 cols ----------
            ind_ps = ips.tile([128, ECH], F32, tag="indps")
            for mc in range(ECH):
                for kc in range(ECH):
                    nc.tensor.matmul(
                        ind_ps[:, mc : mc + 1],
                        lhsT=indt_sb[:, kc, mc * 128 : (mc + 1) * 128],
                        rhs=uvec_sb[:, kc : kc + 1],
                        start=(kc == 0),
                        stop=(kc == ECH - 1),
                    )
            induced_sb = pp.tile([128, ECH], F16, tag="induced")
            nc.vector.tensor_copy(induced_sb[:], ind_ps[:])

            # ---------- gather + transpose -> eT [128, ECH, T] fp16 ----------
            eT = pp.tile([128, ECH, T], F16, tag="eT")
            for c in range(TCH):
                e_c = gp.tile([128, E], F16, tag="echunk")
                nc.gpsimd.indirect_dma_start(
                    out=e_c[:],
                    out_offset=None,
                    in_=tab[:],
                    in_offset=bass.IndirectOffsetOnAxis(ap=tok_sb[:, c : c + 1], axis=0),
                )
                for ec in range(ECH):
                    tp = tps.tile([128, 128], F16, tag="tp")
                    nc.tensor.transpose(
                        out=tp[:], in_=e_c[:, ec * 128 : (ec + 1) * 128], identity=ident[:]
                    )
                    nc.vector.tensor_copy(eT[:, ec, c * 128 : (c + 1) * 128], tp[:])

            # ---------- UNK rank-1 fix: eT += induced (x) unkf ----------
            for ec in range(ECH):
                nc.vector.scalar_tensor_tensor(
                    out=eT[:, ec, :],
                    in0=unkf_sb[:],
                    scalar=induced_sb[:, ec : ec + 1],
                    in1=eT[:, ec, :],
                    op0=OP.mult,
                    op1=OP.add,
                )

            # ---------- xiT = W_ih @ eT (+ biases) -> [128, MCH, T] fp16 ----------
            xiT = pp.tile([128, MCH, T], F16, tag="xiT")
            NT = 512
            for m in range(MCH):
                for q in range(T // NT):
                    xp = xps.tile([128, NT], F32, tag="xp")
                    for ec in range(ECH):
                        nc.tensor.matmul(
                            xp[:],
                            lhsT=wih_sb[:, ec, m * 128 : (m + 1) * 128],
                            rhs=eT[:, ec, q * NT : (q + 1) * NT],
                            start=(ec == 0),
                            stop=(ec == ECH - 1),
                        )
                    nc.vector.tensor_scalar_add(
                        xiT[:, m, q * NT : (q + 1) * NT], xp[:], bsum_sb[:, m : m + 1]
                    )

        # ---------- GRU recurrence (fully unrolled) ----------
        with (
            tc.tile_pool(name="gru_sb", bufs=3) as gsb,
            tc.tile_pool(name="h_pool", bufs=2) as hp,
            tc.tile_pool(name="rz_ps", bufs=2, space="PSUM") as rzp,
            tc.tile_pool(name="n_ps", bufs=2, space="PSUM") as nnp,
            tc.tile_pool(name="z_ps", bufs=2, space="PSUM") as zzp,
            tc.tile_pool(name="zn_ps", bufs=1, space="PSUM") as znp,
            tc.tile_pool(name="fin", bufs=1) as fin,
            tc.tile_pool(name="fin_ps", bufs=1, space="PSUM") as fps,
        ):
            # State runs in g-space: g = h + 1 (corrections host-baked into
            # bsum/bnrep/bproj). n-gate weights are host-scaled by 2 so
            # tanh(x) = 2*sigmoid(2x) - 1 merges into the z sigmoid.
            hT = hp.tile([128, KCH, BL], F16, tag="hT")
            nc.gpsimd.memset(hT[:], 1.0)
            maxT = fin.tile([128, KCH, BL], F16, tag="maxT")
            nc.gpsimd.memset(maxT[:], -1.0e4)

            for t in range(S):
                r_ps = rzp.tile([128, 4, BL], F32, tag="r")
                n_ps = nnp.tile([128, 4, BL], F32, tag="n")
                z_ps = zzp.tile([128, 4, BL], F32, tag="z")
                # PE order r -> n -> z: the r and n gate chains hide under
                # the remaining matmuls; only z's chain trails the PE block.
                for m in range(4):
                    for k in range(KCH):
                        nc.tensor.matmul(
                            r_ps[:, m, :],
                            lhsT=whh_sb[:, k, m * 128 : (m + 1) * 128],
                            rhs=hT[:, k, :],
                            start=(k == 0),
                            stop=(k == KCH - 1),
                        )
                for m in range(8, 12):
                    for k in range(KCH):
                        nc.tensor.matmul(
                            n_ps[:, m - 8, :],
                            lhsT=whh_sb[:, k, m * 128 : (m + 1) * 128],
                            rhs=hT[:, k, :],
                            start=(k == 0),
                            stop=(k == KCH - 1),
                        )
                for m in range(4, 8):
                    for k in range(KCH):
                        nc.tensor.matmul(
                            z_ps[:, m - 4, :],
                            lhsT=whh_sb[:, k, m * 128 : (m + 1) * 128],
                            rhs=hT[:, k, :],
                            start=(k == 0),
                            stop=(k == KCH - 1),
                        )
                # r chain (hidden under n/z matmuls)
                rpre = gsb.tile([128, 4, BL], F32, tag="rpre")
                nc.vector.tensor_add(rpre[:], r_ps[:], xiT[:, 0:4, t * BL : (t + 1) * BL])
                r_s = gsb.tile([128, 4, BL], F16, tag="r_s")
                nc.scalar.activation(r_s[:], rpre[:], AF.Sigmoid)
                # n chain: nb = 2*(hn + bhn) * r  (hidden under z matmuls)
                nb = gsb.tile([128, 4, BL], F32, tag="nb")
                nc.vector.tensor_add(nb[:], n_ps[:], bnrep_sb[:])
                nc.vector.tensor_mul(nb[:], nb[:], r_s[:])
                # merged sigmoid input: [z_pre | 2*n_pre] staged in PSUM
                # (ScalarE PSUM-source reads are ~150ns cheaper than SBUF)
                zn = znp.tile([128, 8, BL], F32, tag="zn")
                i_n2 = nc.vector.tensor_add(
                    zn[:, 4:8, :], nb[:], xiT[:, 8:12, t * BL : (t + 1) * BL]
                )
                i_z = nc.vector.tensor_add(
                    zn[:, 0:4, :], z_ps[:], xiT[:, 4:8, t * BL : (t + 1) * BL]
                )
                # keep the z-gate add (the only op gated on the last matmuls)
                # after the hideable n-chain ops in the DVE queue
                add_dep_helper(i_z.ins, i_n2.ins, sync=False,
                               reason="z-add last on DVE")
                s_sb = gsb.tile([128, 8, BL], F16, tag="s_sb")
                nc.scalar.activation(s_sb[:], zn[:], AF.Sigmoid)
                # g' = 2*s_n + s_z*(g - 2*s_n)
                d_sb = gsb.tile([128, 4, BL], F16, tag="d_sb")
                nc.vector.scalar_tensor_tensor(
                    out=d_sb[:], in0=s_sb[:, 4:8, :], scalar=-2.0, in1=hT[:],
                    op0=OP.mult, op1=OP.add,
                )
                nc.vector.tensor_mul(d_sb[:], d_sb[:], s_sb[:, 0:4, :])
                hT2 = hp.tile([128, KCH, BL], F16, tag="hT")
                nc.vector.scalar_tensor_tensor(
                    out=hT2[:], in0=s_sb[:, 4:8, :], scalar=2.0, in1=d_sb[:],
                    op0=OP.mult, op1=OP.add,
                )
                nc.vector.tensor_max(maxT[:], maxT[:], hT2[:])
                hT = hT2

            # ---------- projection: out = pooled @ W_proj.T + b_proj ----------
            o_ps = fps.tile([BL, C], F32, tag="ops")
            for k in range(KCH):
                nc.tensor.matmul(
                    o_ps[:],
                    lhsT=maxT[:, k, :],
                    rhs=wproj_sb[:, k, :],
                    start=(k == 0),
                    stop=(k == KCH - 1),
                )
            o_sb = fin.tile([BL, C], F32, tag="osb")
            nc.vector.tensor_add(o_sb[:], o_ps[:], bproj_sb[:])
            nc.sync.dma_start(out=out[:], in_=o_sb[:])

    nc.compile()
    return nc


def _prep_inputs(x, emb_table, unk_vec, induction, W_ih, W_hh, b_ih, b_hh, W_proj, b_proj):
    """Host-side marshalling: shard over batch, pack layouts, cast to fp16."""
    x = np.asarray(x)
    tok = np.where(x == -1, VOCAB, x).astype(np.int32)       # [B, S]
    unk = (x == -1).astype(np.float16)                        # [B, S]

    tab16 = np.asarray(emb_table).astype(np.float16)          # [V+1, E]
    W_ih = np.asarray(W_ih).astype(np.float32)
    W_hh = np.asarray(W_hh).astype(np.float32)
    wih_s = W_ih.copy()
    wih_s[2 * H :, :] *= 2.0                                  # n gate x2 (tanh->sigmoid)
    whh_s = W_hh.copy()
    whh_s[2 * H :, :] *= 2.0
    wih16 = wih_s.T.astype(np.float16).copy()                 # [E, 3H]
    whh16 = whh_s.T.astype(np.float16).copy()                 # [H, 3H]
    whh_rowsum = W_hh.sum(axis=1).astype(np.float32)          # (W_hh @ 1)[g], unscaled
    indt16 = np.asarray(induction).T.astype(np.float16).copy()  # [E, E] (k=j major)
    uv16 = np.asarray(unk_vec).astype(np.float16).reshape(ECH, 128).T.copy()  # [128, ECH]
    b_ih = np.asarray(b_ih).astype(np.float32)
    b_hh = np.asarray(b_hh).astype(np.float32)
    bihT = b_ih.reshape(MCH, 128).T                           # [128, 12]
    bhhT = b_hh.reshape(MCH, 128).T
    csT = whh_rowsum.reshape(MCH, 128).T                      # g-space correction
    bsum = bihT.copy()
    bsum[:, 0:8] += bhhT[:, 0:8] - csT[:, 0:8]                # r,z: biases - W_hh@1
    bsum[:, 8:12] *= 2.0                                      # n: 2*b_ih (no b_hh here)
    bsum = np.ascontiguousarray(bsum, dtype=np.float32)
    bn = 2.0 * (bhhT[:, 8:12] - csT[:, 8:12])                 # n: 2*(b_hh - W_hh@1)
    bnrep = np.repeat(bn[:, :, None], BL, axis=2).reshape(128, 4 * BL)
    bnrep = np.ascontiguousarray(bnrep, dtype=np.float32)
    W_proj = np.asarray(W_proj).astype(np.float32)
    wproj16 = W_proj.T.astype(np.float16).copy()              # [H, C]
    bp = np.asarray(b_proj).astype(np.float32).reshape(1, C) - W_proj.sum(axis=1)[None, :]
    bproj32 = np.repeat(bp, BL, axis=0)                       # b - W_proj@1 (pooled in g-space)

    shared = dict(
        tab=tab16, wih=wih16, whh=whh16, bsum=bsum, bnrep=bnrep,
        indt=indt16, uvec=uv16, wproj=wproj16, bproj=bproj32,
    )
    in_maps = []
    for i in range(NCORES):
        tok_i = tok[i * BL : (i + 1) * BL]                    # [BL, S]
        unk_i = unk[i * BL : (i + 1) * BL]
        tflat = tok_i.T.reshape(-1)                           # s-major, t = s*BL + b
        uflat = unk_i.T.reshape(-1)
        tokp = np.ascontiguousarray(tflat.reshape(TCH, 128).T, dtype=np.int32)
        unkf = np.ascontiguousarray(
            np.repeat(uflat[None, :], 128, axis=0), dtype=np.float16
        )
        in_maps.append(dict(shared, tokp=tokp, unkf=unkf))
    return in_maps


def _ensure_trace_hook():
    """Best-effort: make trace=True usable under axon.

    bass_utils fetches the NTFF hook from ``antenv.axon_hooks``; some agent
    images lack that module (boot degrades silently). Shim the registry and
    register the ctypes hook on libaxon_pjrt.so ourselves when possible.
    """
    import contextlib
    import ctypes
    import sys
    import types

    try:
        try:
            from antenv import axon_hooks  # noqa: PLC0415
        except ImportError:
            import antenv  # noqa: PLC0415

            axon_hooks = types.ModuleType("antenv.axon_hooks")
            _hook_box = [None]
            axon_hooks.set_axon_ntff_profile_hook = lambda h: _hook_box.__setitem__(0, h)
            axon_hooks.get_axon_ntff_profile_hook = lambda: _hook_box[0]
            sys.modules["antenv.axon_hooks"] = axon_hooks
            antenv.axon_hooks = axon_hooks
        if axon_hooks.get_axon_ntff_profile_hook() is not None:
            return True
        so_path = "/opt/axon/libaxon_pjrt.so"
        lib = ctypes.CDLL(so_path)
        if not hasattr(lib, "axon_start_nrt_profile"):
            return False
        lib.axon_start_nrt_profile.argtypes = [
            ctypes.POINTER(ctypes.c_int64),
            ctypes.c_size_t,
        ]
        lib.axon_start_nrt_profile.restype = ctypes.c_int64
        lib.axon_stop_nrt_profile.argtypes = [ctypes.c_char_p]
        lib.axon_stop_nrt_profile.restype = ctypes.c_int64

        @contextlib.contextmanager
        def _hook(output_dir, device_ids):
            import jax  # noqa: PLC0415

            jax.devices()
            if device_ids:
                ids = (ctypes.c_int64 * len(device_ids))(*device_ids)
                rc = lib.axon_start_nrt_profile(ids, len(device_ids))
            else:
                rc = lib.axon_start_nrt_profile(None, 0)
            if rc != 0:
                raise RuntimeError(f"axon_start_nrt_profile rc={rc}")
            try:
                yield
            finally:
                n = lib.axon_stop_nrt_profile(str(output_dir).encode())
                if n < 0:
                    raise RuntimeError(f"axon_stop_nrt_profile rc={n}")

        axon_hooks.set_axon_ntff_profile_hook(_hook)
        return True
    except Exception:
        return False


def kernel(**inputs):
    global LAST_RESULT
    import os

    nc = build_nc()
    in_maps = _prep_inputs(**inputs)
    trace = os.environ.get("KERNEL_TRACE", "1") == "1"
    if trace:
        trace = _ensure_trace_hook()
    core_ids = list(range(NCORES))
    try:
        res = run_bass_kernel_spmd(nc, in_maps, core_ids=core_ids, trace=trace)
    except Exception:
        if not trace:
            raise
        res = run_bass_kernel_spmd(nc, in_maps, core_ids=core_ids, trace=False)
    LAST_RESULT = res
    out = np.concatenate([r["out"] for r in res.results], axis=0)  # [B, C]
    return out.astype(np.float32)


# revision 12
# speedup vs baseline: 1.2757x; 1.0010x over previous
"""ALaCarteClassifier Trainium2 kernel.

Model: embedding gather -> UNK substitution -> GRU(S=512,H=512) -> maxpool -> linear.
Sharding: data-parallel over batch (B=32) across 8 NeuronCores (4 rows/core).
Embedding table + weights replicated per core. No collectives.

Device pipeline per core (B_loc=4, T=2048 tokens, s-major token order t=s*4+b):
  1. indirect-DMA gather of fp16 table rows -> e [tok(part), 256]   (memory-bound part)
  2. PE-transpose e -> eT [e-dim(part), 2, T]; UNK fix as rank-1 update
     eT += induced (x) unkf  (one scalar_tensor_tensor per e-chunk)
  3. xiT[g, tok] = W_ih @ eT + (b_ih + b_hh baked for r,z; b_ih for n)  via PE
  4. GRU recurrence, 512 fully-unrolled steps; stationary fp16 W_hh tiles (FWL),
     moving hT [128,4]; gates in PSUM; running max-pool on DVE
  5. pooled @ W_proj.T + b_proj via PSUM accumulation (ones (x) b trick not needed:
     b added via DVE broadcast); DMA out [4, 2] f32
"""

import numpy as np

import concourse.bass as bass
import concourse.mybir as mybir
import concourse.tile as tile
from concourse import bacc
from concourse.bass_utils import run_bass_kernel_spmd
from concourse.masks import make_identity
from concourse.tile_rust import add_dep_helper

# problem dims (hardcoded per harness rules)
VOCAB = 200000
E = 256
H = 512
B = 32
S = 512
C = 2
NCORES = 8
BL = B // NCORES          # 4 batch rows per core
T = BL * S                # 2048 tokens per core
TCH = T // 128            # 16 token chunks
ECH = E // 128            # 2 embedding-dim chunks
KCH = H // 128            # 4 hidden-dim chunks (GRU contraction)
MCH = 3 * H // 128        # 12 gate-row chunks (r:0-3, z:4-7, n:8-11)

F16 = mybir.dt.float16
F32 = mybir.dt.float32
I32 = mybir.dt.int32
AF = mybir.ActivationFunctionType
OP = mybir.AluOpType

# exposed for test.py
LAST_RESULT = None


def build_nc():
    nc = bacc.Bacc("TRN2", target_bir_lowering=False, debug=False, num_devices=NCORES)

    # ---- DRAM parameters (per-core shards / replicated weights) ----
    tab = nc.declare_dram_parameter("tab", [VOCAB + 1, E], F16, isOutput=False)
    tokp = nc.declare_dram_parameter("tokp", [128, TCH], I32, isOutput=False)
    unkf = nc.declare_dram_parameter("unkf", [128, T], F16, isOutput=False)
    wih = nc.declare_dram_parameter("wih", [E, 3 * H], F16, isOutput=False)
    whh = nc.declare_dram_parameter("whh", [H, 3 * H], F16, isOutput=False)
    bsum = nc.declare_dram_parameter("bsum", [128, MCH], F32, isOutput=False)
    bnrep = nc.declare_dram_parameter("bnrep", [128, 4 * BL], F32, isOutput=False)
    indt = nc.declare_dram_parameter("indt", [E, E], F16, isOutput=False)
    uvec = nc.declare_dram_parameter("uvec", [128, ECH], F16, isOutput=False)
    wproj = nc.declare_dram_parameter("wproj", [H, C], F16, isOutput=False)
    bproj = nc.declare_dram_parameter("bproj", [BL, C], F32, isOutput=False)
    out = nc.declare_dram_parameter("out", [BL, C], F32, isOutput=True)

    with tile.TileContext(nc) as tc:
        with (
            tc.tile_pool(name="persist", bufs=1) as pp,
            tc.tile_pool(name="gather", bufs=TCH) as gp,
            tc.tile_pool(name="tpsum", bufs=2, space="PSUM") as tps,
            tc.tile_pool(name="ipsum", bufs=1, space="PSUM") as ips,
            tc.tile_pool(name="xpsum", bufs=4, space="PSUM") as xps,
            tc.tile_pool(name="work", bufs=3) as wp,
        ):
            # ---------- load weights / metadata ----------
            wih_sb = pp.tile([128, ECH, 3 * H], F16, tag="wih")
            nc.sync.dma_start(out=wih_sb[:], in_=wih.rearrange("(c p) g -> p c g", p=128))
            whh_sb = pp.tile([128, KCH, 3 * H], F16, tag="whh")
            nc.sync.dma_start(out=whh_sb[:], in_=whh.rearrange("(c p) g -> p c g", p=128))
            tok_sb = pp.tile([128, TCH], I32, tag="tok")
            nc.sync.dma_start(out=tok_sb[:], in_=tokp[:])
            unkf_sb = pp.tile([128, T], F16, tag="unkf")
            nc.sync.dma_start(out=unkf_sb[:], in_=unkf[:])
            bsum_sb = pp.tile([128, MCH], F32, tag="bsum")
            nc.sync.dma_start(out=bsum_sb[:], in_=bsum[:])
            bnrep_sb = pp.tile([128, 4 * BL], F32, tag="bnrep")
            nc.sync.dma_start(out=bnrep_sb[:], in_=bnrep[:])
            indt_sb = pp.tile([128, ECH, E], F16, tag="indt")
            nc.sync.dma_start(out=indt_sb[:], in_=indt.rearrange("(c p) g -> p c g", p=128))
            uvec_sb = pp.tile([128, ECH], F16, tag="uvec")
            nc.sync.dma_start(out=uvec_sb[:], in_=uvec[:])
            wproj_sb = pp.tile([128, KCH, C], F16, tag="wproj")
            nc.sync.dma_start(out=wproj_sb[:], in_=wproj.rearrange("(c p) n -> p c n", p=128))
            bproj_sb = pp.tile([BL, C], F32, tag="bproj")
            nc.sync.dma_start(out=bproj_sb[:], in_=bproj[:])

            ident = pp.tile([128, 128], F16, tag="ident")
            make_identity(nc, ident[:])

            # ---------- induced = induction @ unk_vec  -> [128, ECH] fp16 cols ----------
            ind_ps = ips.tile([128, ECH], F32, tag="indps")
            for mc in range(ECH):
                for kc in range(ECH):
                    nc.tensor.matmul(
                        ind_ps[:, mc : mc + 1],
                        lhsT=indt_sb[:, kc, mc * 128 : (mc + 1) * 128],
                        rhs=uvec_sb[:, kc : kc + 1],
                        start=(kc == 0),
                        stop=(kc == ECH - 1),
                    )
            induced_sb = pp.tile([128, ECH], F16, tag="induced")
            nc.vector.tensor_copy(induced_sb[:], ind_ps[:])

            # ---------- gather + transpose -> eT [128, ECH, T] fp16 ----------
            eT = pp.tile([128, ECH, T], F16, tag="eT")
            for c in range(TCH):
                e_c = gp.tile([128, E], F16, tag="echunk")
                nc.gpsimd.indirect_dma_start(
                    out=e_c[:],
                    out_offset=None,
                    in_=tab[:],
                    in_offset=bass.IndirectOffsetOnAxis(ap=tok_sb[:, c : c + 1], axis=0),
                )
                for ec in range(ECH):
                    tp = tps.tile([128, 128], F16, tag="tp")
                    nc.tensor.transpose(
                        out=tp[:], in_=e_c[:, ec * 128 : (ec + 1) * 128], identity=ident[:]
                    )
                    nc.vector.tensor_copy(eT[:, ec, c * 128 : (c + 1) * 128], tp[:])

            # ---------- UNK rank-1 fix: eT += induced (x) unkf ----------
            for ec in range(ECH):
                nc.vector.scalar_tensor_tensor(
                    out=eT[:, ec, :],
                    in0=unkf_sb[:],
                    scalar=induced_sb[:, ec : ec + 1],
                    in1=eT[:, ec, :],
                    op0=OP.mult,
                    op1=OP.add,
                )

            # ---------- xiT = W_ih @ eT (+ biases), step-major layout ----------
            # xiT[p, t, m*BL + b]: per-step slices are flat contiguous APs
            xiT = pp.tile([128, S, MCH * BL], F16, tag="xiT")
            NT = 512
            for m in range(MCH):
                for q in range(T // NT):
                    xp = xps.tile([128, NT], F32, tag="xp")
                    for ec in range(ECH):
                        nc.tensor.matmul(
                            xp[:],
                            lhsT=wih_sb[:, ec, m * 128 : (m + 1) * 128],
                            rhs=eT[:, ec, q * NT : (q + 1) * NT],
                            start=(ec == 0),
                            stop=(ec == ECH - 1),
                        )
                    tsl = slice(q * (NT // BL), (q + 1) * (NT // BL))
                    nc.vector.tensor_scalar_add(
                        xiT[:, tsl, m * BL : (m + 1) * BL],
                        xp[:].rearrange("p (t b) -> p t b", b=BL),
                        bsum_sb[:, m : m + 1],
                    )

        # ---------- GRU recurrence (fully unrolled) ----------
        with (
            tc.tile_pool(name="gru_sb", bufs=3) as gsb,
            tc.tile_pool(name="h_pool", bufs=2) as hp,
            tc.tile_pool(name="rz_ps", bufs=2, space="PSUM") as rzp,
            tc.tile_pool(name="n_ps", bufs=2, space="PSUM") as nnp,
            tc.tile_pool(name="z_ps", bufs=2, space="PSUM") as zzp,
            tc.tile_pool(name="zn_ps", bufs=1, space="PSUM") as znp,
            tc.tile_pool(name="fin", bufs=1) as fin,
            tc.tile_pool(name="fin_ps", bufs=1, space="PSUM") as fps,
        ):
            # State runs in g-space: g = h + 1 (corrections host-baked into
            # bsum/bnrep/bproj). n-gate weights are host-scaled by 2 so
            # tanh(x) = 2*sigmoid(2x) - 1 merges into the z sigmoid.
            # All GRU tiles are flat [128, N] so every AP is 1-D contiguous.
            hT = hp.tile([128, KCH * BL], F16, tag="hT")
            nc.gpsimd.memset(hT[:], 1.0)
            maxT = fin.tile([128, KCH * BL], F16, tag="maxT")
            nc.gpsimd.memset(maxT[:], -1.0e4)

            for t in range(S):
                r_ps = rzp.tile([128, 4 * BL], F32, tag="r")
                n_ps = nnp.tile([128, 4 * BL], F32, tag="n")
                z_ps = zzp.tile([128, 4 * BL], F32, tag="z")
                # PE order r -> n -> z: the r and n gate chains hide under
                # the remaining matmuls; only z's chain trails the PE block.
                for m in range(4):
                    for k in range(KCH):
                        nc.tensor.matmul(
                            r_ps[:, m * BL : (m + 1) * BL],
                            lhsT=whh_sb[:, k, m * 128 : (m + 1) * 128],
                            rhs=hT[:, k * BL : (k + 1) * BL],
                            start=(k == 0),
                            stop=(k == KCH - 1),
                        )
                for m in range(8, 12):
                    for k in range(KCH):
                        nc.tensor.matmul(
                            n_ps[:, (m - 8) * BL : (m - 7) * BL],
                            lhsT=whh_sb[:, k, m * 128 : (m + 1) * 128],
                            rhs=hT[:, k * BL : (k + 1) * BL],
                            start=(k == 0),
                            stop=(k == KCH - 1),
                        )
                for m in range(4, 8):
                    for k in range(KCH):
                        nc.tensor.matmul(
                            z_ps[:, (m - 4) * BL : (m - 3) * BL],
                            lhsT=whh_sb[:, k, m * 128 : (m + 1) * 128],
                            rhs=hT[:, k * BL : (k + 1) * BL],
                            start=(k == 0),
                            stop=(k == KCH - 1),
                        )
                # r chain (hidden under n/z matmuls)
                rpre = gsb.tile([128, 4 * BL], F32, tag="rpre")
                nc.vector.tensor_add(rpre[:], r_ps[:], xiT[:, t, 0 : 4 * BL])
                r_s = gsb.tile([128, 4 * BL], F16, tag="r_s")
                nc.scalar.activation(r_s[:], rpre[:], AF.Sigmoid)
                # n chain: nb = 2*(hn + bhn) * r  (hidden under z matmuls)
                nb = gsb.tile([128, 4 * BL], F32, tag="nb")
                nc.vector.tensor_add(nb[:], n_ps[:], bnrep_sb[:])
                nc.vector.tensor_mul(nb[:], nb[:], r_s[:])
                # merged sigmoid input: [z_pre | 2*n_pre] staged in PSUM
                # (ScalarE PSUM-source reads are cheaper than SBUF)
                zn = znp.tile([128, 8 * BL], F32, tag="zn")
                i_n2 = nc.vector.tensor_add(
                    zn[:, 4 * BL : 8 * BL], nb[:], xiT[:, t, 8 * BL : 12 * BL]
                )
                i_z = nc.vector.tensor_add(
                    zn[:, 0 : 4 * BL], z_ps[:], xiT[:, t, 4 * BL : 8 * BL]
                )
                # keep the z-gate add (the only op gated on the last matmuls)
                # after the hideable n-chain ops in the DVE queue
                add_dep_helper(i_z.ins, i_n2.ins, sync=False,
                               reason="z-add last on DVE")
                s_sb = gsb.tile([128, 8 * BL], F16, tag="s_sb")
                nc.scalar.activation(s_sb[:], zn[:], AF.Sigmoid)
                # g' = 2*s_n + s_z*(g - 2*s_n)
                d_sb = gsb.tile([128, 4 * BL], F16, tag="d_sb")
                nc.vector.scalar_tensor_tensor(
                    out=d_sb[:], in0=s_sb[:, 4 * BL : 8 * BL], scalar=-2.0, in1=hT[:],
                    op0=OP.mult, op1=OP.add,
                )
                nc.vector.tensor_mul(d_sb[:], d_sb[:], s_sb[:, 0 : 4 * BL])
                hT2 = hp.tile([128, KCH * BL], F16, tag="hT")
                nc.vector.scalar_tensor_tensor(
                    out=hT2[:], in0=s_sb[:, 4 * BL : 8 * BL], scalar=2.0, in1=d_sb[:],
                    op0=OP.mult, op1=OP.add,
                )
                nc.vector.tensor_max(maxT[:], maxT[:], hT2[:])
                hT = hT2

            # ---------- projection: out = pooled @ W_proj.T + b_proj ----------
            o_ps = fps.tile([BL, C], F32, tag="ops")
            for k in range(KCH):
                nc.tensor.matmul(
                    o_ps[:],
                    lhsT=maxT[:, k * BL : (k + 1) * BL],
                    rhs=wproj_sb[:, k, :],
                    start=(k == 0),
                    stop=(k == KCH - 1),
                )
            o_sb = fin.tile([BL, C], F32, tag="osb")
            nc.vector.tensor_add(o_sb[:], o_ps[:], bproj_sb[:])
            nc.sync.dma_start(out=out[:], in_=o_sb[:])

    nc.compile()
    return nc


def _prep_inputs(x, emb_table, unk_vec, induction, W_ih, W_hh, b_ih, b_hh, W_proj, b_proj):
    """Host-side marshalling: shard over batch, pack layouts, cast to fp16."""
    x = np.asarray(x)
    tok = np.where(x == -1, VOCAB, x).astype(np.int32)       # [B, S]
    unk = (x == -1).astype(np.float16)                        # [B, S]

    tab16 = np.asarray(emb_table).astype(np.float16)          # [V+1, E]
    W_ih = np.asarray(W_ih).astype(np.float32)
    W_hh = np.asarray(W_hh).astype(np.float32)
    wih_s = W_ih.copy()
    wih_s[2 * H :, :] *= 2.0                                  # n gate x2 (tanh->sigmoid)
    whh_s = W_hh.copy()
    whh_s[2 * H :, :] *= 2.0
    wih16 = wih_s.T.astype(np.float16).copy()                 # [E, 3H]
    whh16 = whh_s.T.astype(np.float16).copy()                 # [H, 3H]
    whh_rowsum = W_hh.sum(axis=1).astype(np.float32)          # (W_hh @ 1)[g], unscaled
    indt16 = np.asarray(induction).T.astype(np.float16).copy()  # [E, E] (k=j major)
    uv16 = np.asarray(unk_vec).astype(np.float16).reshape(ECH, 128).T.copy()  # [128, ECH]
    b_ih = np.asarray(b_ih).astype(np.float32)
    b_hh = np.asarray(b_hh).astype(np.float32)
    bihT = b_ih.reshape(MCH, 128).T                           # [128, 12]
    bhhT = b_hh.reshape(MCH, 128).T
    csT = whh_rowsum.reshape(MCH, 128).T                      # g-space correction
    bsum = bihT.copy()
    bsum[:, 0:8] += bhhT[:, 0:8] - csT[:, 0:8]                # r,z: biases - W_hh@1
    bsum[:, 8:12] *= 2.0                                      # n: 2*b_ih (no b_hh here)
    bsum = np.ascontiguousarray(bsum, dtype=np.float32)
    bn = 2.0 * (bhhT[:, 8:12] - csT[:, 8:12])                 # n: 2*(b_hh - W_hh@1)
    bnrep = np.repeat(bn[:, :, None], BL, axis=2).reshape(128, 4 * BL)
    bnrep = np.ascontiguousarray(bnrep, dtype=np.float32)
    W_proj = np.asarray(W_proj).astype(np.float32)
    wproj16 = W_proj.T.astype(np.float16).copy()              # [H, C]
    bp = np.asarray(b_proj).astype(np.float32).reshape(1, C) - W_proj.sum(axis=1)[None, :]
    bproj32 = np.repeat(bp, BL, axis=0)                       # b - W_proj@1 (pooled in g-space)

    shared = dict(
        tab=tab16, wih=wih16, whh=whh16, bsum=bsum, bnrep=bnrep,
        indt=indt16, uvec=uv16, wproj=wproj16, bproj=bproj32,
    )
    in_maps = []
    for i in range(NCORES):
        tok_i = tok[i * BL : (i + 1) * BL]                    # [BL, S]
        unk_i = unk[i * BL : (i + 1) * BL]
        tflat = tok_i.T.reshape(-1)                           # s-major, t = s*BL + b
        uflat = unk_i.T.reshape(-1)
        tokp = np.ascontiguousarray(tflat.reshape(TCH, 128).T, dtype=np.int32)
        unkf = np.ascontiguousarray(
            np.repeat(uflat[None, :], 128, axis=0), dtype=np.float16
        )
        in_maps.append(dict(shared, tokp=tokp, unkf=unkf))
    return in_maps


def _ensure_trace_hook():
    """Best-effort: make trace=True usable under axon.

    bass_utils fetches the NTFF hook from ``antenv.axon_hooks``; some agent
    images lack that module (boot degrades silently). Shim the registry and
    register the ctypes hook on libaxon_pjrt.so ourselves when possible.
    """
    import contextlib
    import ctypes
    import sys
    import types

    try:
        try:
            from antenv import axon_hooks  # noqa: PLC0415
        except ImportError:
            import antenv  # noqa: PLC0415

            axon_hooks = types.ModuleType("antenv.axon_hooks")
            _hook_box = [None]
            axon_hooks.set_axon_ntff_profile_hook = lambda h: _hook_box.__setitem__(0, h)
            axon_hooks.get_axon_ntff_profile_hook = lambda: _hook_box[0]
            sys.modules["antenv.axon_hooks"] = axon_hooks
            antenv.axon_hooks = axon_hooks
        if axon_hooks.get_axon_ntff_profile_hook() is not None:
            return True
        so_path = "/opt/axon/libaxon_pjrt.so"
        lib = ctypes.CDLL(so_path)
        if not hasattr(lib, "axon_start_nrt_profile"):
            return False
        lib.axon_start_nrt_profile.argtypes = [
            ctypes.POINTER(ctypes.c_int64),
            ctypes.c_size_t,
        ]
        lib.axon_start_nrt_profile.restype = ctypes.c_int64
        lib.axon_stop_nrt_profile.argtypes = [ctypes.c_char_p]
        lib.axon_stop_nrt_profile.restype = ctypes.c_int64

        @contextlib.contextmanager
        def _hook(output_dir, device_ids):
            import jax  # noqa: PLC0415

            jax.devices()
            if device_ids:
                ids = (ctypes.c_int64 * len(device_ids))(*device_ids)
                rc = lib.axon_start_nrt_profile(ids, len(device_ids))
            else:
                rc = lib.axon_start_nrt_profile(None, 0)
            if rc != 0:
                raise RuntimeError(f"axon_start_nrt_profile rc={rc}")
            try:
                yield
            finally:
                n = lib.axon_stop_nrt_profile(str(output_dir).encode())
                if n < 0:
                    raise RuntimeError(f"axon_stop_nrt_profile rc={n}")

        axon_hooks.set_axon_ntff_profile_hook(_hook)
        return True
    except Exception:
        return False


def kernel(**inputs):
    global LAST_RESULT
    import os

    nc = build_nc()
    in_maps = _prep_inputs(**inputs)
    trace = os.environ.get("KERNEL_TRACE", "1") == "1"
    if trace:
        trace = _ensure_trace_hook()
    core_ids = list(range(NCORES))
    try:
        res = run_bass_kernel_spmd(nc, in_maps, core_ids=core_ids, trace=trace)
    except Exception:
        if not trace:
            raise
        res = run_bass_kernel_spmd(nc, in_maps, core_ids=core_ids, trace=False)
    LAST_RESULT = res
    out = np.concatenate([r["out"] for r in res.results], axis=0)  # [B, C]
    return out.astype(np.float32)


# revision 15
# speedup vs baseline: 1.3665x; 1.0712x over previous
"""ALaCarteClassifier Trainium2 kernel.

Model: embedding gather -> UNK substitution -> GRU(S=512,H=512) -> maxpool -> linear.
Sharding: data-parallel over batch (B=32) across 8 NeuronCores (4 rows/core).
Embedding table + weights replicated per core. No collectives.

Device pipeline per core (B_loc=4, T=2048 tokens, s-major token order t=s*4+b):
  1. indirect-DMA gather of fp16 table rows -> e [tok(part), 256]   (memory-bound part)
  2. PE-transpose e -> eT [e-dim(part), 2, T]; UNK fix as rank-1 update
     eT += induced (x) unkf  (one scalar_tensor_tensor per e-chunk)
  3. xiT[g, tok] = W_ih @ eT + (b_ih + b_hh baked for r,z; b_ih for n)  via PE
  4. GRU recurrence, 512 fully-unrolled steps; stationary fp16 W_hh tiles (FWL),
     moving hT [128,4]; gates in PSUM; running max-pool on DVE
  5. pooled @ W_proj.T + b_proj via PSUM accumulation (ones (x) b trick not needed:
     b added via DVE broadcast); DMA out [4, 2] f32
"""

import numpy as np

import concourse.bass as bass
import concourse.mybir as mybir
import concourse.tile as tile
from concourse import bacc
from concourse.bass_utils import run_bass_kernel_spmd
from concourse.masks import make_identity
from concourse.tile_rust import add_dep_helper

# problem dims (hardcoded per harness rules)
VOCAB = 200000
E = 256
H = 512
B = 32
S = 512
C = 2
NCORES = 8
BL = B // NCORES          # 4 batch rows per core
T = BL * S                # 2048 tokens per core
TCH = T // 128            # 16 token chunks
ECH = E // 128            # 2 embedding-dim chunks
KCH = H // 128            # 4 hidden-dim chunks (GRU contraction)
MCH = 3 * H // 128        # 12 gate-row chunks (r:0-3, z:4-7, n:8-11)

F16 = mybir.dt.float16
F32 = mybir.dt.float32
I32 = mybir.dt.int32
AF = mybir.ActivationFunctionType
OP = mybir.AluOpType

# exposed for test.py
LAST_RESULT = None


def build_nc():
    nc = bacc.Bacc("TRN2", target_bir_lowering=False, debug=False, num_devices=NCORES)

    # ---- DRAM parameters (per-core shards / replicated weights) ----
    tab = nc.declare_dram_parameter("tab", [VOCAB + 1, E], F16, isOutput=False)
    tokp = nc.declare_dram_parameter("tokp", [128, TCH], I32, isOutput=False)
    unkf = nc.declare_dram_parameter("unkf", [128, T], F16, isOutput=False)
    wih = nc.declare_dram_parameter("wih", [E, 3 * H], F16, isOutput=False)
    whh = nc.declare_dram_parameter("whh", [H, 3 * H], F16, isOutput=False)
    bsum = nc.declare_dram_parameter("bsum", [128, MCH], F32, isOutput=False)
    bnrep = nc.declare_dram_parameter("bnrep", [128, 4 * BL], F16, isOutput=False)
    ideye = nc.declare_dram_parameter("ideye", [128, 128], F16, isOutput=False)
    indt = nc.declare_dram_parameter("indt", [E, E], F16, isOutput=False)
    uvec = nc.declare_dram_parameter("uvec", [128, ECH], F16, isOutput=False)
    wproj = nc.declare_dram_parameter("wproj", [H, C], F16, isOutput=False)
    bproj = nc.declare_dram_parameter("bproj", [BL, C], F32, isOutput=False)
    out = nc.declare_dram_parameter("out", [BL, C], F32, isOutput=True)

    with tile.TileContext(nc) as tc:
        with (
            tc.tile_pool(name="persist", bufs=1) as pp,
            tc.tile_pool(name="gather", bufs=TCH) as gp,
            tc.tile_pool(name="tpsum", bufs=2, space="PSUM") as tps,
            tc.tile_pool(name="ipsum", bufs=1, space="PSUM") as ips,
            tc.tile_pool(name="xpsum", bufs=4, space="PSUM") as xps,
            tc.tile_pool(name="work", bufs=3) as wp,
        ):
            # ---------- load weights / metadata ----------
            wih_sb = pp.tile([128, ECH, 3 * H], F16, tag="wih")
            nc.sync.dma_start(out=wih_sb[:], in_=wih.rearrange("(c p) g -> p c g", p=128))
            whh_sb = pp.tile([128, KCH, 3 * H], F16, tag="whh")
            nc.sync.dma_start(out=whh_sb[:], in_=whh.rearrange("(c p) g -> p c g", p=128))
            tok_sb = pp.tile([128, TCH], I32, tag="tok")
            nc.sync.dma_start(out=tok_sb[:], in_=tokp[:])
            unkf_sb = pp.tile([128, T], F16, tag="unkf")
            nc.sync.dma_start(out=unkf_sb[:], in_=unkf[:])
            bsum_sb = pp.tile([128, MCH], F32, tag="bsum")
            nc.sync.dma_start(out=bsum_sb[:], in_=bsum[:])
            bnrep_sb = pp.tile([128, 4 * BL], F16, tag="bnrep")
            nc.sync.dma_start(out=bnrep_sb[:], in_=bnrep[:])
            eye_sb = pp.tile([128, 128], F16, tag="eye")
            nc.sync.dma_start(out=eye_sb[:], in_=ideye[:])
            indt_sb = pp.tile([128, ECH, E], F16, tag="indt")
            nc.sync.dma_start(out=indt_sb[:], in_=indt.rearrange("(c p) g -> p c g", p=128))
            uvec_sb = pp.tile([128, ECH], F16, tag="uvec")
            nc.sync.dma_start(out=uvec_sb[:], in_=uvec[:])
            wproj_sb = pp.tile([128, KCH, C], F16, tag="wproj")
            nc.sync.dma_start(out=wproj_sb[:], in_=wproj.rearrange("(c p) n -> p c n", p=128))
            bproj_sb = pp.tile([BL, C], F32, tag="bproj")
            nc.sync.dma_start(out=bproj_sb[:], in_=bproj[:])

            ident = pp.tile([128, 128], F16, tag="ident")
            make_identity(nc, ident[:])

            # ---------- induced = induction @ unk_vec  -> [128, ECH] fp16 cols ----------
            ind_ps = ips.tile([128, ECH], F32, tag="indps")
            for mc in range(ECH):
                for kc in range(ECH):
                    nc.tensor.matmul(
                        ind_ps[:, mc : mc + 1],
                        lhsT=indt_sb[:, kc, mc * 128 : (mc + 1) * 128],
                        rhs=uvec_sb[:, kc : kc + 1],
                        start=(kc == 0),
                        stop=(kc == ECH - 1),
                    )
            induced_sb = pp.tile([128, ECH], F16, tag="induced")
            nc.vector.tensor_copy(induced_sb[:], ind_ps[:])

            # ---------- gather + transpose -> eT [128, ECH, T] fp16 ----------
            eT = pp.tile([128, ECH, T], F16, tag="eT")
            for c in range(TCH):
                e_c = gp.tile([128, E], F16, tag="echunk")
                nc.gpsimd.indirect_dma_start(
                    out=e_c[:],
                    out_offset=None,
                    in_=tab[:],
                    in_offset=bass.IndirectOffsetOnAxis(ap=tok_sb[:, c : c + 1], axis=0),
                )
                for ec in range(ECH):
                    tp = tps.tile([128, 128], F16, tag="tp")
                    nc.tensor.transpose(
                        out=tp[:], in_=e_c[:, ec * 128 : (ec + 1) * 128], identity=ident[:]
                    )
                    nc.vector.tensor_copy(eT[:, ec, c * 128 : (c + 1) * 128], tp[:])

            # ---------- UNK rank-1 fix: eT += induced (x) unkf ----------
            for ec in range(ECH):
                nc.vector.scalar_tensor_tensor(
                    out=eT[:, ec, :],
                    in0=unkf_sb[:],
                    scalar=induced_sb[:, ec : ec + 1],
                    in1=eT[:, ec, :],
                    op0=OP.mult,
                    op1=OP.add,
                )

            # ---------- xiT = W_ih @ eT (+ biases), step-major layout ----------
            # xiT[p, t, m*BL + b]: per-step slices are flat contiguous APs
            xiT = pp.tile([128, S, MCH * BL], F16, tag="xiT")
            NT = 512
            for m in range(MCH):
                for q in range(T // NT):
                    xp = xps.tile([128, NT], F32, tag="xp")
                    for ec in range(ECH):
                        nc.tensor.matmul(
                            xp[:],
                            lhsT=wih_sb[:, ec, m * 128 : (m + 1) * 128],
                            rhs=eT[:, ec, q * NT : (q + 1) * NT],
                            start=(ec == 0),
                            stop=(ec == ECH - 1),
                        )
                    tsl = slice(q * (NT // BL), (q + 1) * (NT // BL))
                    nc.vector.tensor_scalar_add(
                        xiT[:, tsl, m * BL : (m + 1) * BL],
                        xp[:].rearrange("p (t b) -> p t b", b=BL),
                        bsum_sb[:, m : m + 1],
                    )

        # ---------- GRU recurrence (fully unrolled) ----------
        with (
            tc.tile_pool(name="gru_sb", bufs=3) as gsb,
            tc.tile_pool(name="h_pool", bufs=2) as hp,
            tc.tile_pool(name="rz_ps", bufs=2, space="PSUM") as rzp,
            tc.tile_pool(name="n_ps", bufs=2, space="PSUM") as nnp,
            tc.tile_pool(name="z_ps", bufs=2, space="PSUM") as zzp,
            tc.tile_pool(name="zn_ps", bufs=1, space="PSUM") as znp,
            tc.tile_pool(name="fin", bufs=1) as fin,
            tc.tile_pool(name="fin_ps", bufs=1, space="PSUM") as fps,
        ):
            # State runs in g-space: g = h + 1 (corrections host-baked into
            # bsum/bnrep/bproj). n-gate weights are host-scaled by 2 so
            # tanh(x) = 2*sigmoid(2x) - 1 merges into the z sigmoid.
            # All GRU tiles are flat [128, N] so every AP is 1-D contiguous.
            hT = hp.tile([128, KCH * BL], F16, tag="hT")
            nc.gpsimd.memset(hT[:], 1.0)
            maxT = fin.tile([128, KCH * BL], F16, tag="maxT")
            nc.gpsimd.memset(maxT[:], -1.0e4)

            for t in range(S):
                r_ps = rzp.tile([128, 4 * BL], F32, tag="r")
                n_ps = nnp.tile([128, 4 * BL], F32, tag="n")
                z_ps = zzp.tile([128, 4 * BL], F32, tag="z")
                # PE order r -> n -> z: the r and n gate chains hide under
                # the remaining matmuls; only z's chain trails the PE block.
                for m in range(4):
                    for k in range(KCH):
                        nc.tensor.matmul(
                            r_ps[:, m * BL : (m + 1) * BL],
                            lhsT=whh_sb[:, k, m * 128 : (m + 1) * 128],
                            rhs=hT[:, k * BL : (k + 1) * BL],
                            start=(k == 0),
                            stop=(k == KCH - 1),
                        )
                # bias seeded on PE first (start=True clears whole-bank
                # has_written bits, so the seed must be the first write);
                # the W matmuls then accumulate onto it. Frees the DVE queue.
                nc.tensor.matmul(
                    n_ps[:], lhsT=eye_sb[:], rhs=bnrep_sb[:], start=True, stop=False
                )
                for m in range(8, 12):
                    for k in range(KCH):
                        nc.tensor.matmul(
                            n_ps[:, (m - 8) * BL : (m - 7) * BL],
                            lhsT=whh_sb[:, k, m * 128 : (m + 1) * 128],
                            rhs=hT[:, k * BL : (k + 1) * BL],
                            start=False,
                            stop=(m == 11 and k == KCH - 1),
                        )
                for m in range(4, 8):
                    for k in range(KCH):
                        nc.tensor.matmul(
                            z_ps[:, (m - 4) * BL : (m - 3) * BL],
                            lhsT=whh_sb[:, k, m * 128 : (m + 1) * 128],
                            rhs=hT[:, k * BL : (k + 1) * BL],
                            start=(k == 0),
                            stop=(k == KCH - 1),
                        )
                # r chain (hidden under n/z matmuls)
                rpre = gsb.tile([128, 4 * BL], F32, tag="rpre")
                nc.vector.tensor_add(rpre[:], r_ps[:], xiT[:, t, 0 : 4 * BL])
                r_s = gsb.tile([128, 4 * BL], F16, tag="r_s")
                nc.scalar.activation(r_s[:], rpre[:], AF.Sigmoid)
                # n chain: nb = 2*(hn + bhn) * r  (hidden under z matmuls)
                nb = gsb.tile([128, 4 * BL], F32, tag="nb")
                nc.vector.tensor_mul(nb[:], n_ps[:], r_s[:])
                # merged sigmoid input: [z_pre | 2*n_pre] staged in PSUM
                # (ScalarE PSUM-source reads are cheaper than SBUF)
                zn = znp.tile([128, 8 * BL], F32, tag="zn")
                i_n2 = nc.vector.tensor_add(
                    zn[:, 4 * BL : 8 * BL], nb[:], xiT[:, t, 8 * BL : 12 * BL]
                )
                i_z = nc.vector.tensor_add(
                    zn[:, 0 : 4 * BL], z_ps[:], xiT[:, t, 4 * BL : 8 * BL]
                )
                # keep the z-gate add (the only op gated on the last matmuls)
                # after the hideable n-chain ops in the DVE queue
                add_dep_helper(i_z.ins, i_n2.ins, sync=False,
                               reason="z-add last on DVE")
                s_sb = gsb.tile([128, 8 * BL], F16, tag="s_sb")
                nc.scalar.activation(s_sb[:], zn[:], AF.Sigmoid)
                # g' = 2*s_n + s_z*(g - 2*s_n)
                d_sb = gsb.tile([128, 4 * BL], F16, tag="d_sb")
                nc.vector.scalar_tensor_tensor(
                    out=d_sb[:], in0=s_sb[:, 4 * BL : 8 * BL], scalar=-2.0, in1=hT[:],
                    op0=OP.mult, op1=OP.add,
                )
                nc.vector.tensor_mul(d_sb[:], d_sb[:], s_sb[:, 0 : 4 * BL])
                hT2 = hp.tile([128, KCH * BL], F16, tag="hT")
                nc.vector.scalar_tensor_tensor(
                    out=hT2[:], in0=s_sb[:, 4 * BL : 8 * BL], scalar=2.0, in1=d_sb[:],
                    op0=OP.mult, op1=OP.add,
                )
                nc.vector.tensor_max(maxT[:], maxT[:], hT2[:])
                hT = hT2

            # ---------- projection: out = pooled @ W_proj.T + b_proj ----------
            o_ps = fps.tile([BL, C], F32, tag="ops")
            for k in range(KCH):
                nc.tensor.matmul(
                    o_ps[:],
                    lhsT=maxT[:, k * BL : (k + 1) * BL],
                    rhs=wproj_sb[:, k, :],
                    start=(k == 0),
                    stop=(k == KCH - 1),
                )
            o_sb = fin.tile([BL, C], F32, tag="osb")
            nc.vector.tensor_add(o_sb[:], o_ps[:], bproj_sb[:])
            nc.sync.dma_start(out=out[:], in_=o_sb[:])

    nc.compile()
    return nc


def _prep_inputs(x, emb_table, unk_vec, induction, W_ih, W_hh, b_ih, b_hh, W_proj, b_proj):
    """Host-side marshalling: shard over batch, pack layouts, cast to fp16."""
    x = np.asarray(x)
    tok = np.where(x == -1, VOCAB, x).astype(np.int32)       # [B, S]
    unk = (x == -1).astype(np.float16)                        # [B, S]

    tab16 = np.asarray(emb_table).astype(np.float16)          # [V+1, E]
    W_ih = np.asarray(W_ih).astype(np.float32)
    W_hh = np.asarray(W_hh).astype(np.float32)
    wih_s = W_ih.copy()
    wih_s[2 * H :, :] *= 2.0                                  # n gate x2 (tanh->sigmoid)
    whh_s = W_hh.copy()
    whh_s[2 * H :, :] *= 2.0
    wih16 = wih_s.T.astype(np.float16).copy()                 # [E, 3H]
    whh16 = whh_s.T.astype(np.float16).copy()                 # [H, 3H]
    whh_rowsum = W_hh.sum(axis=1).astype(np.float32)          # (W_hh @ 1)[g], unscaled
    indt16 = np.asarray(induction).T.astype(np.float16).copy()  # [E, E] (k=j major)
    uv16 = np.asarray(unk_vec).astype(np.float16).reshape(ECH, 128).T.copy()  # [128, ECH]
    b_ih = np.asarray(b_ih).astype(np.float32)
    b_hh = np.asarray(b_hh).astype(np.float32)
    bihT = b_ih.reshape(MCH, 128).T                           # [128, 12]
    bhhT = b_hh.reshape(MCH, 128).T
    csT = whh_rowsum.reshape(MCH, 128).T                      # g-space correction
    bsum = bihT.copy()
    bsum[:, 0:8] += bhhT[:, 0:8] - csT[:, 0:8]                # r,z: biases - W_hh@1
    bsum[:, 8:12] *= 2.0                                      # n: 2*b_ih (no b_hh here)
    bsum = np.ascontiguousarray(bsum, dtype=np.float32)
    bn = 2.0 * (bhhT[:, 8:12] - csT[:, 8:12])                 # n: 2*(b_hh - W_hh@1)
    bnrep = np.repeat(bn[:, :, None], BL, axis=2).reshape(128, 4 * BL)
    bnrep = np.ascontiguousarray(bnrep, dtype=np.float16)
    W_proj = np.asarray(W_proj).astype(np.float32)
    wproj16 = W_proj.T.astype(np.float16).copy()              # [H, C]
    bp = np.asarray(b_proj).astype(np.float32).reshape(1, C) - W_proj.sum(axis=1)[None, :]
    bproj32 = np.repeat(bp, BL, axis=0)                       # b - W_proj@1 (pooled in g-space)

    shared = dict(
        tab=tab16, wih=wih16, whh=whh16, bsum=bsum, bnrep=bnrep,
        indt=indt16, uvec=uv16, wproj=wproj16, bproj=bproj32,
        ideye=np.eye(128, dtype=np.float16),
    )
    in_maps = []
    for i in range(NCORES):
        tok_i = tok[i * BL : (i + 1) * BL]                    # [BL, S]
        unk_i = unk[i * BL : (i + 1) * BL]
        tflat = tok_i.T.reshape(-1)                           # s-major, t = s*BL + b
        uflat = unk_i.T.reshape(-1)
        tokp = np.ascontiguousarray(tflat.reshape(TCH, 128).T, dtype=np.int32)
        unkf = np.ascontiguousarray(
            np.repeat(uflat[None, :], 128, axis=0), dtype=np.float16
        )
        in_maps.append(dict(shared, tokp=tokp, unkf=unkf))
    return in_maps


def _ensure_trace_hook():
    """Best-effort: make trace=True usable under axon.

    bass_utils fetches the NTFF hook from ``antenv.axon_hooks``; some agent
    images lack that module (boot degrades silently). Shim the registry and
    register the ctypes hook on libaxon_pjrt.so ourselves when possible.
    """
    import contextlib
    import ctypes
    import sys
    import types

    try:
        try:
            from antenv import axon_hooks  # noqa: PLC0415
        except ImportError:
            import antenv  # noqa: PLC0415

            axon_hooks = types.ModuleType("antenv.axon_hooks")
            _hook_box = [None]
            axon_hooks.set_axon_ntff_profile_hook = lambda h: _hook_box.__setitem__(0, h)
            axon_hooks.get_axon_ntff_profile_hook = lambda: _hook_box[0]
            sys.modules["antenv.axon_hooks"] = axon_hooks
            antenv.axon_hooks = axon_hooks
        if axon_hooks.get_axon_ntff_profile_hook() is not None:
            return True
        so_path = "/opt/axon/libaxon_pjrt.so"
        lib = ctypes.CDLL(so_path)
        if not hasattr(lib, "axon_start_nrt_profile"):
            return False
        lib.axon_start_nrt_profile.argtypes = [
            ctypes.POINTER(ctypes.c_int64),
            ctypes.c_size_t,
        ]
        lib.axon_start_nrt_profile.restype = ctypes.c_int64
        lib.axon_stop_nrt_profile.argtypes = [ctypes.c_char_p]
        lib.axon_stop_nrt_profile.restype = ctypes.c_int64

        @contextlib.contextmanager
        def _hook(output_dir, device_ids):
            import jax  # noqa: PLC0415

            jax.devices()
            if device_ids:
                ids = (ctypes.c_int64 * len(device_ids))(*device_ids)
                rc = lib.axon_start_nrt_profile(ids, len(device_ids))
            else:
                rc = lib.axon_start_nrt_profile(None, 0)
            if rc != 0:
                raise RuntimeError(f"axon_start_nrt_profile rc={rc}")
            try:
                yield
            finally:
                n = lib.axon_stop_nrt_profile(str(output_dir).encode())
                if n < 0:
                    raise RuntimeError(f"axon_stop_nrt_profile rc={n}")

        axon_hooks.set_axon_ntff_profile_hook(_hook)
        return True
    except Exception:
        return False


def kernel(**inputs):
    global LAST_RESULT
    import os

    nc = build_nc()
    in_maps = _prep_inputs(**inputs)
    trace = os.environ.get("KERNEL_TRACE", "1") == "1"
    if trace:
        trace = _ensure_trace_hook()
    core_ids = list(range(NCORES))
    try:
        res = run_bass_kernel_spmd(nc, in_maps, core_ids=core_ids, trace=trace)
    except Exception:
        if not trace:
            raise
        res = run_bass_kernel_spmd(nc, in_maps, core_ids=core_ids, trace=False)
    LAST_RESULT = res
    out = np.concatenate([r["out"] for r in res.results], axis=0)  # [B, C]
    return out.astype(np.float32)
